# revision 48
# baseline (speedup 1.0000x reference)
"""Trainium2 Bass kernel for a 4-layer dense transformer (AKT-style).

Sharding: data-parallel over batch. B=8 batch elements -> 1 per NeuronCore.
Each core runs the full 4-layer stack on its own (S=1024, D=512) slice with
no collectives; weights are replicated.

Per-core layout: feature-major activations [D, S] in bf16 (partition dim =
feature tiles of 128). The residual stream lives entirely in bf16 (x_bf =
post-LN, x2_bf = pre-LN residual); LN statistics come from PE ones-matmul
sums of the same bf16 tensors, so stats and apply are consistent. Matmuls
run in bf16 with fp32 PSUM accumulation. Attention uses the symmetric-scores
trick: S = kq @ kq^T is symmetric, so a [j, i]-layout strip of scores doubles
as the transposed-probabilities operand after a strictly-upper-triangular
causal mask; softmax denominators come from an extra ones-column appended to
V. The per-(head, i) normalizer is computed as a bf16 reciprocal straight
from the PSUM sums row, broadcast across 64 partitions (DRAM-bounce DMA for
heads 0..5, local K=1 PE matmul + SBUF copy for the last two heads where the
DMA round-trip would be exposed), and folded into the PSUM->SBUF eviction
multiply — TT ops may read at most one PSUM operand, so the broadcast side
lives in SBUF. Engine notes: sync=HWDGE carries startup loads, bounces, and
output stores in emission order (weight prefetch queues behind the bounces
on purpose); gpsimd=SWDGE only carries latency-tolerant params. Next-layer
v-projections are spread as PE filler under the softmax-exp (Act-bound) and
LN-stats chains.
"""
import sys

sys.path.insert(0, "/opt/trn_rl_repo")

import math

import ml_dtypes
import numpy as np

import concourse.bass as bass
import concourse.tile as tile
from concourse import bacc, mybir
from concourse.bass_utils import run_bass_kernel_spmd

F32 = mybir.dt.float32
BF16 = mybir.dt.bfloat16
F8 = mybir.dt.float8e4
DR = mybir.MatmulPerfMode.DoubleRow
AF = mybir.ActivationFunctionType
ALU = mybir.AluOpType

# fp8 split scales (powers of 2). Weights are scaled up so both the hi part
# and the hi-lo residual stay clear of the e4m3 subnormal floor; activations
# (x, y, h1) are O(1)..O(32) and need no scaling. psum descale happens at
# eviction.
WV_SCL = 32.0   # v-proj psum = 32 * v
W2_SCL = 64.0   # ffn2 psum = 64 * ff (h1 hi/lo is stored at natural scale)

B, S, D, H, FF, L = 8, 1024, 512, 8, 2048, 4
DK = D // H          # 64
NKT = D // 128       # 4  feature tiles
NJT = S // 128       # 8  token tiles
NFT = FF // 128      # 16 ffn tiles
SCALE = 1.0 / math.sqrt(DK)
EPS = 1e-5
NCORES = 8

_PROG_CACHE = {}


def _strip_chunks(a):
    """Column chunks (absolute i ranges) for scores/PV strip of j-tile a:
    the 128-wide diagonal block first, then pieces that cross neither an
    absolute 512-boundary (PV psum banks) nor a strip-local one (scores
    psum banks, local = absolute - 128*a)."""
    chunks = [(128 * a, 128 * a + 128)]
    start = 128 * a + 128
    pts = sorted({512, 128 * a + 512, S})
    for p in pts:
        if start < p <= S:
            chunks.append((start, p))
            start = p
    return chunks


def _build(has_bv, bk_zero=True, ln1_triv=True, ln2_triv=True, b2_zero=True):
    nc = bacc.Bacc("TRN2", target_bir_lowering=False, debug=False,
                   num_devices=NCORES)

    xT_e = nc.declare_dram_parameter("xT", [D, S], BF16, isOutput=False)
    yh_e = nc.declare_dram_parameter("yTh", [D, S], F8, isOutput=False)
    yl_e = nc.declare_dram_parameter("yTl", [D, S], F8, isOutput=False)
    wk_e = nc.declare_dram_parameter("wkT", [L, D, D], BF16, isOutput=False)
    wvh_e = nc.declare_dram_parameter("wvTh", [L, D, D], F8, isOutput=False)
    wvl_e = nc.declare_dram_parameter("wvTl", [L, D, D], F8, isOutput=False)
    wo_e = nc.declare_dram_parameter("woT", [L, D, D], BF16, isOutput=False)
    w1_e = nc.declare_dram_parameter("w1T", [L, D, FF], BF16, isOutput=False)
    w2h_e = nc.declare_dram_parameter("w2Th", [L, FF, D], F8, isOutput=False)
    w2l_e = nc.declare_dram_parameter("w2Tl", [L, FF, D], F8, isOutput=False)
    # per-feature params packed [128, L, ntiles]
    bk_e = nc.declare_dram_parameter("bkp", [128, L, NKT], F32, isOutput=False)
    bo_e = nc.declare_dram_parameter("bop", [128, L, NKT], F32, isOutput=False)
    b1_e = nc.declare_dram_parameter("b1p", [128, L, NFT], F32, isOutput=False)
    b2_e = nc.declare_dram_parameter("b2p", [128, L, NKT], F32, isOutput=False)
    l1s_e = nc.declare_dram_parameter("l1s", [128, L, NKT], F32, isOutput=False)
    l1b_e = nc.declare_dram_parameter("l1b", [128, L, NKT], F32, isOutput=False)
    l2s_e = nc.declare_dram_parameter("l2s", [128, L, NKT], F32, isOutput=False)
    l2b_e = nc.declare_dram_parameter("l2b", [128, L, NKT], F32, isOutput=False)
    bv_e = nc.declare_dram_parameter("bvp", [1, L, D], BF16, isOutput=False) if has_bv else None
    mask_e = nc.declare_dram_parameter("mask01", [128, 128], BF16, isOutput=False)
    out_e = nc.declare_dram_parameter("outT", [D, S], F32, isOutput=True)

    with tile.TileContext(nc) as tc:
        with (
            tc.tile_pool(name="res", bufs=1) as res,         # resident activations
            tc.tile_pool(name="wqkv", bufs=2) as wqkv,       # per-layer D x D weights
            tc.tile_pool(name="wff", bufs=2) as wff,         # per-layer ffn weights
            tc.tile_pool(name="pt", bufs=5) as ptp,          # exp'd prob strips
            tc.tile_pool(name="vp", bufs=2) as vp,           # v_ext double buffer
            tc.tile_pool(name="rc", bufs=3) as rcp,          # per-head recip rows
            tc.tile_pool(name="bc", bufs=1) as bc,           # LN broadcast tiles
            tc.tile_pool(name="tb", bufs=2) as tbp,
            tc.tile_pool(name="h1p", bufs=1) as h1p,
            tc.tile_pool(name="wff2", bufs=1) as wff2,          # LN apply temps
            tc.tile_pool(name="ps", bufs=2, space="PSUM") as ps,
            tc.tile_pool(name="psh", bufs=4, space="PSUM") as psh,
            tc.tile_pool(name="rb", bufs=3) as rbp,          # recip bcast rows
            tc.tile_pool(name="dr", bufs=4, space="DRAM") as dr,
        ):
            # ---- residents (bf16 residual stream)
            x_bf = res.tile([128, NKT, S], BF16, tag="x_bf")     # post-LN x
            x2_bf = res.tile([128, NKT, S], BF16, tag="x2_bf")   # pre-LN resid
            y_hi = res.tile([128, NKT, S], F8, tag="y_hi")
            y_lo = res.tile([128, NKT, S], F8, tag="y_lo")
            kq_bf = res.tile([128, NKT, S], BF16, tag="kq_bf")
            outcat = res.tile([128, NKT, S], BF16, tag="outcat")
            # h1 fp8 hi/lo, one tile per 512-token chunk: keeps the Act-relu
            # (hi) and DVE-stt (lo) evictions of different chunks free of
            # false subtile WARs (tracking is first-free-dim granular)
            h1_hi = [h1p.tile([128, NFT, 512], F8, tag=f"h1h{c}", name=f"h1h{c}")
                     for c in range(2)]
            h1_lo = [h1p.tile([128, NFT, 512], F8, tag=f"h1l{c}", name=f"h1l{c}")
                     for c in range(2)]
            xsq_bf = res.tile([128, NKT, S], BF16, tag="xsq")

            # ---- initial loads: x = q+pe, y = qa+pe precomputed on host.
            # sync = HWDGE (fast, low latency): carries the startup-critical
            # loads in need order (y+wv feed vproj first), then only the
            # latency-sensitive softmax bounces + output stores.
            # gpsimd = SWDGE (Q7 descgen ~1us, latency-tolerant): params,
            # mask, and all next-layer weight prefetches.
            yh4 = yh_e.rearrange("(k p) s -> p k s", p=128)
            yl4 = yl_e.rearrange("(k p) s -> p k s", p=128)

            def ln_stats_mm(lp, kt, ch):
                """One accumulation step of the per-chunk stats sums; emit
                kt lagged behind the producer's evict of feature tile kt so
                it never head-of-line-blocks the producer's own matmuls."""
                lp0, lp1 = lp
                cs = slice(ch * 512, ch * 512 + 512)
                nc.tensor.matmul(lp0, lhsT=ones128, rhs=x2_bf[:, kt, cs],
                                 start=(kt == 0), stop=(kt == NKT - 1))
                nc.tensor.matmul(lp1, lhsT=ones128, rhs=xsq_bf[:, kt, cs],
                                 start=(kt == 0), stop=(kt == NKT - 1))

            def new_lp():
                return (psh.tile([128, 512], F32, tag="mmh", name="lp0"),
                        psh.tile([128, 512], F32, tag="mmh", name="lp1"))

            def ln_stats_fin(lp, ch, meanb, sbv):
                """Finish one chunk's stats: mean/E[x2] eviction split across
                Act (mean, mean^2) and DVE (E[x2], var) so the serial chain is
                ~1us shorter than an all-DVE version; rstd via Ln/Exp."""
                lp0, lp1 = lp
                cs = slice(ch * 512, ch * 512 + 512)
                nc.scalar.mul(meanb[:, cs], lp0, 1.0 / D)
                nc.vector.tensor_scalar_mul(sbv[:, cs], lp1, 1.0 / D)
                m2 = tbp.tile([128, 512], BF16, tag="lntmp")
                nc.scalar.square(m2, meanb[:, cs])
                nc.vector.scalar_tensor_tensor(out=sbv[:, cs], in0=sbv[:, cs],
                                               scalar=float(EPS), in1=m2,
                                               op0=ALU.add, op1=ALU.subtract)
                nc.scalar.activation(out=sbv[:, cs], in_=sbv[:, cs], func=AF.Ln)
                nc.scalar.activation(out=sbv[:, cs], in_=sbv[:, cs],
                                     func=AF.Exp, scale=-0.5)

            def ln_stats_ch(ch, meanb, sbv):
                lp = new_lp()
                for kt in range(NKT):
                    ln_stats_mm(lp, kt, ch)
                ln_stats_fin(lp, ch, meanb, sbv)

            def new_stats():
                meanb = bc.tile([128, S], BF16, tag="meanb")
                sbv = bc.tile([128, S], BF16, tag="statb")  # ex2->var->rstd
                return meanb, sbv

            oute4 = out_e.rearrange("(k p) s -> p k s", p=128)

            def ln_apply_ch(ch, meanb, sbv, lname_s, lname_b, li, triv,
                            final=False, xout_ch=None):
                """Apply one chunk's LN: x = (x2 - mean) * rstd (+affine).
                Non-final+trivial uses 2 coarse DVE ops; final writes the
                fp32 staging tile and DMAs per kt."""
                cs = slice(ch * 512, ch * 512 + 512)
                if not final and triv:
                    # pairs of kt per op: x(kt0,kt1) lands early enough for
                    # the consumer matmuls' first accumulation steps while
                    # keeping the DVE queue short
                    for kt in range(0, NKT, 2):
                        mb = meanb[:, None, cs].to_broadcast([128, 2, 512])
                        sb = sbv[:, None, cs].to_broadcast([128, 2, 512])
                        nc.vector.tensor_sub(xsq_bf[:, kt:kt + 2, cs],
                                             x2_bf[:, kt:kt + 2, cs], mb)
                        nc.vector.tensor_mul(x_bf[:, kt:kt + 2, cs],
                                             xsq_bf[:, kt:kt + 2, cs], sb)
                    return
                for kt in range(NKT):
                    nc.vector.tensor_sub(xsq_bf[:, kt, cs],
                                         x2_bf[:, kt, cs], meanb[:, cs])
                    if final:
                        po = xout_ch[:, kt, :]
                        nc.vector.tensor_mul(po, xsq_bf[:, kt, cs], sbv[:, cs])
                        if not triv:
                            nc.vector.tensor_scalar(
                                out=po, in0=po,
                                scalar1=params[lname_s][:, li, kt:kt + 1],
                                scalar2=params[lname_b][:, li, kt:kt + 1],
                                op0=ALU.mult, op1=ALU.add)
                        nc.sync.dma_start(out=oute4[:, kt, cs], in_=po)
                    else:
                        nc.vector.tensor_mul(x_bf[:, kt, cs],
                                             xsq_bf[:, kt, cs], sbv[:, cs])
                        if not triv:
                            nc.vector.tensor_scalar(
                                out=x_bf[:, kt, cs], in0=x_bf[:, kt, cs],
                                scalar1=params[lname_s][:, li, kt:kt + 1],
                                scalar2=params[lname_b][:, li, kt:kt + 1],
                                op0=ALU.mult, op1=ALU.add)

            def layernorm(lname_s, lname_b, li, triv, filler=None, stats=None):
                """LN over features of x2_bf -> x_bf. Stats from PE
                ones-matmul sums of the same bf16 values the apply uses; pass
                `stats`=(meanb, sbv) when the producing phase already emitted
                per-ch stats."""
                if stats is None:
                    meanb, sbv = new_stats()
                    for ch in range(2):
                        ln_stats_ch(ch, meanb, sbv)
                else:
                    meanb, sbv = stats
                if filler is not None:
                    filler()
                for ch in range(2):
                    ln_apply_ch(ch, meanb, sbv, lname_s, lname_b, li, triv)


            def vproj(li, wv, jts):
                """v = y @ WvT (token-major) -- depends only on y hi/lo + wv,
                so it can fill PE bubbles in other phases. fp8 DoubleRow,
                3-term split: psum holds 32*v, descaled at eviction."""
                wvh, wvl = wv
                vx4 = _vx4_of[li]
                nc.gpsimd.memset(vx4[:, jts.start:jts.stop, :, 64:65], 1.0)
                for jt in jts:
                    pp = ps.tile([128, S], F32, tag="mm")
                    for half in range(2):
                        c0 = half * 256
                        sl = pp[:, c0:c0 + 256]
                        seq = [(y_hi, wvh, 0), (y_hi, wvh, 2),
                               (y_hi, wvl, 0), (y_hi, wvl, 2),
                               (y_lo, wvh, 0), (y_lo, wvh, 2)]
                        for i, (ya, wa, ktp) in enumerate(seq):
                            nc.tensor.matmul(
                                sl, lhsT=ya[:, ktp:ktp + 2, jt * 128:jt * 128 + 128],
                                rhs=wa[:, ktp:ktp + 2, c0:c0 + 256],
                                start=(i == 0),
                                stop=(i == len(seq) - 1) and not has_bv,
                                perf_mode=DR)
                        if has_bv:
                            nc.tensor.matmul(sl, lhsT=ones_row,
                                             rhs=bv_sb[0:1, li, c0:c0 + 256],
                                             start=False, stop=True,
                                             skip_group_check=True)
                    nc.scalar.mul(
                        vx4[:, jt, :, 0:64],
                        pp[:, 0:512].rearrange("p (h c) -> p h c", c=64),
                        1.0 / WV_SCL)
                return vx4

            _vx4_of = {}

            def new_vext(li):
                vext = vp.tile([128, NJT, H * 72], BF16, tag="vext")
                _vx4_of[li] = vext.rearrange("p j (h c) -> p j h c", c=72)
                return _vx4_of[li]

            wv_tiles = {}
            wk_tiles = {}
            wo_tiles = {}

            def load_wv(li, eng):
                th = wqkv.tile([128, NKT, D], F8, tag="wvh")
                eng.dma_start(out=th, in_=wvh_e[li].rearrange("(k p) m -> p k m", p=128))
                tl = wqkv.tile([128, NKT, D], F8, tag="wvl")
                eng.dma_start(out=tl, in_=wvl_e[li].rearrange("(k p) m -> p k m", p=128))
                return th, tl

            def load_wk_wo(li, eng):
                wk = wqkv.tile([128, NKT, D], BF16, tag="wk")
                eng.dma_start(out=wk, in_=wk_e[li].rearrange("(k p) m -> p k m", p=128))
                wo = wqkv.tile([128, NKT, D], BF16, tag="wo")
                eng.dma_start(out=wo, in_=wo_e[li].rearrange("(k p) m -> p k m", p=128))
                return wk, wo

            wff_tiles = {}

            def load_wff(li, eng):
                w1 = wff.tile([128, NKT, FF], BF16, tag="w1")
                eng.dma_start(out=w1, in_=w1_e[li].rearrange("(k p) m -> p k m", p=128))
                w2h = wff2.tile([128, NFT, D], F8, tag="w2h")
                eng.dma_start(out=w2h, in_=w2h_e[li].rearrange("(k p) m -> p k m", p=128))
                w2l = wff2.tile([128, NFT, D], F8, tag="w2l")
                eng.dma_start(out=w2l, in_=w2l_e[li].rearrange("(k p) m -> p k m", p=128))
                return w1, (w2h, w2l)

            nc.sync.dma_start(out=y_hi[:, :, 0:512], in_=yh4[:, :, 0:512])
            nc.sync.dma_start(out=y_lo[:, :, 0:512], in_=yl4[:, :, 0:512])
            wv_tiles[0] = load_wv(0, nc.sync)
            nc.sync.dma_start(out=y_hi[:, :, 512:1024], in_=yh4[:, :, 512:1024])
            nc.sync.dma_start(out=y_lo[:, :, 512:1024], in_=yl4[:, :, 512:1024])
            xT4 = xT_e.rearrange("(k p) s -> p k s", p=128)
            nc.sync.dma_start(out=x_bf[:, :, 0:512], in_=xT4[:, :, 0:512])
            nc.sync.dma_start(out=x_bf[:, :, 512:1024], in_=xT4[:, :, 512:1024])
            wk_tiles[0], wo_tiles[0] = load_wk_wo(0, nc.sync)
            wff_tiles[0] = load_wff(0, nc.sync)

            # ---- constants & params (issued after the critical-path loads)
            params = {}
            mask01 = res.tile([128, 128], BF16, tag="mask")
            nc.gpsimd.dma_start(out=mask01, in_=mask_e[:])
            ones128 = res.tile([128, 128], BF16, tag="ones")
            nc.vector.memset(ones128, 1.0)
            ones_1x64 = res.tile([1, 64], BF16, tag="ones64")
            nc.vector.memset(ones_1x64, 1.0)
            if has_bv:
                ones_row = res.tile([1, 128], BF16, tag="onesr")
                nc.vector.memset(ones_row, 1.0)
                bv_sb = res.tile([1, L, D], BF16, tag="bv")
                nc.gpsimd.dma_start(out=bv_sb, in_=bv_e[:])
            for name, ext, nt in (("bk", bk_e, NKT), ("bo", bo_e, NKT),
                                  ("b1", b1_e, NFT), ("b2", b2_e, NKT),
                                  ("l1s", l1s_e, NKT), ("l1b", l1b_e, NKT),
                                  ("l2s", l2s_e, NKT), ("l2b", l2b_e, NKT)):
                t = res.tile([128, L, nt], F32, tag="prm_" + name)
                nc.gpsimd.dma_start(out=t, in_=ext[:])
                params[name] = t

            new_vext(0)
            vx4 = vproj(0, wv_tiles[0], range(0, NJT))

            for li in range(L):
                wk, wo = wk_tiles[li], wo_tiles[li]
                w1, (w2h, w2l) = wff_tiles[li]
                if li + 1 < L:
                    new_vext(li + 1)
                    # wv early on the same HWDGE queue: its transfer lands
                    # during kq-proj, before the first softmax bounce
                    wv_tiles[li + 1] = load_wv(li + 1, nc.sync)

                # ---- kq projection tile mt feeds heads 2mt, 2mt+1:
                # interleave so exps start after one kq tile and later kq
                # tiles keep the PE busy under the early heads' exps
                def kq_tile(mt):
                    pp = ps.tile([128, S], F32, tag="mm")
                    for ch in range(2):
                        c0 = ch * 512
                        for kt in range(NKT):
                            nc.tensor.matmul(pp[:, c0:c0 + 512],
                                             lhsT=wk[:, kt, mt * 128:mt * 128 + 128],
                                             rhs=x_bf[:, kt, c0:c0 + 512],
                                             start=(kt == 0), stop=(kt == NKT - 1))
                    # DVE eviction: keeps the Act queue clear so head-0 exps
                    # aren't stuck behind three unrelated kq evictions
                    if bk_zero:
                        nc.vector.tensor_copy(kq_bf[:, mt, :], pp)
                    else:
                        nc.vector.tensor_scalar_add(
                            kq_bf[:, mt, :], pp, params["bk"][:, li, mt:mt + 1])

                for mt in range(NKT):
                    kq_tile(mt)

                # ---- attention, head by head
                for h in range(H):
                    po = (h % 2) * 64
                    kqh = kq_bf[po:po + 64, h // 2, :]
                    acc0 = psh.tile([128, 512], F32, tag="mmh")
                    acc1 = psh.tile([128, 512], F32, tag="mmh")
                    # strip groups share one psum tile + one exp: strips
                    # 4..7 are narrow enough to pack pairwise with no padding
                    # (local offsets chosen so no matmul crosses a psum bank)
                    def emit_group(group, locs):
                        sc = ps.tile([128, S], F32, tag="mm")
                        for a in group:
                            for (s0, e0) in _strip_chunks(a):
                                lo = locs[a] + s0 - 128 * a
                                nc.tensor.matmul(sc[:, lo:lo + (e0 - s0)],
                                                 lhsT=kqh[:, 128 * a:128 * a + 128],
                                                 rhs=kqh[:, s0:s0 + (e0 - s0)],
                                                 start=True, stop=True)
                        wtot = max(locs[a] + S - 128 * a for a in group)
                        pt = ptp.tile([128, S], BF16, tag="pt")
                        nc.scalar.activation(out=pt[:, 0:wtot], in_=sc[:, 0:wtot],
                                             func=AF.Exp, scale=float(SCALE))
                        if (len(group) == 2
                                and locs[group[1]] - locs[group[0]] <= 512):
                            stride = locs[group[1]] - locs[group[0]]
                            view = pt[:, 0:stride * len(group)].rearrange(
                                "p (g c) -> p g c", c=stride)
                            nc.vector.tensor_mul(view[:, :, 0:128], view[:, :, 0:128],
                                                 mask01[:, None, :].to_broadcast(
                                                     [128, len(group), 128]))
                        else:
                            for a in group:
                                lo = locs[a]
                                nc.vector.tensor_mul(pt[:, lo:lo + 128],
                                                     pt[:, lo:lo + 128], mask01)
                        return pt

                    def emit_pv(group, locs, pt):
                        # PV needs only the diagonal + absolute-512 splits
                        # (strip-local splits are a scores-psum constraint)
                        for a in group:
                            chunks = [(128 * a, 128 * a + 128)]
                            st = 128 * a + 128
                            for p in (512, S):
                                if st < p <= S:
                                    chunks.append((st, p))
                                    st = p
                            for (s0, e0) in chunks:
                                acc = acc0 if s0 < 512 else acc1
                                o0 = s0 - (0 if s0 < 512 else 512)
                                lo = locs[a] + s0 - 128 * a
                                nc.tensor.matmul(acc[0:65, o0:o0 + (e0 - s0)],
                                                 lhsT=vx4[:, a, h, 0:65],
                                                 rhs=pt[:, lo:lo + (e0 - s0)],
                                                 start=(a == 0),
                                                 stop=(e0 == 128 * a + 128),
                                                 skip_group_check=True)

                    # softmax denominators: bf16 reciprocal straight from the
                    # psum ones-row, col 0 zeroed (empty first query row), then
                    # a DRAM-bounce DMA (HWDGE, low latency) broadcasts it
                    # across 64 partitions and the eviction multiply
                    # normalizes straight out of psum (TT ops may read at most
                    # ONE operand from PSUM, so the broadcast side must be
                    # SBUF). acc0 (cols 0..512) finishes one PV group before
                    # acc1, so its bounce overlaps the last PV group.
                    recip = rcp.tile([1, S], BF16, tag="recip")
                    bnc = dr.tile([1, S], BF16, tag="bounce")
                    rbh = rbp.tile([64, S], BF16, tag="rbh")

                    def norm_half(c0):
                        acc = acc0 if c0 == 0 else acc1
                        cs = slice(c0, c0 + 512)
                        with nc.allow_low_precision(reason="softmax 1/l bf16"):
                            nc.vector.reciprocal(recip[:, cs], acc[64:65, :])
                        if c0 == 0:
                            nc.gpsimd.memset(recip[:, 0:1], 0.0)
                        if h >= H - 2:
                            # last head: local broadcast (PE matmul -> psum
                            # -> SBUF copy) instead of the DRAM bounce; the
                            # ~5us DMA round-trip would be fully exposed at
                            # the phase boundary
                            bps = psh.tile([128, 512], F32, tag="mmh")
                            nc.tensor.matmul(bps[0:64, :], lhsT=ones_1x64,
                                             rhs=recip[0:1, cs],
                                             start=True, stop=True)
                            nc.vector.tensor_copy(rbh[:, cs], bps[0:64, :])
                        else:
                            nc.sync.dma_start(out=bnc[:, cs], in_=recip[:, cs])
                            bap = bnc[:, cs]
                            nc.sync.dma_start(out=rbh[:, cs], in_=bass.AP(
                                tensor=bap.tensor, offset=bap.offset,
                                ap=[[0, 64]] + bap.ap[1:]))

                    def evict_half(c0):
                        acc = acc0 if c0 == 0 else acc1
                        cs = slice(c0, c0 + 512)
                        nc.vector.tensor_mul(outcat[po:po + 64, h // 2, cs],
                                             acc[0:64, :], rbh[:, cs])

                    GROUPS = [([0], {0: 0}), ([1], {1: 0}),
                              ([2, 6], {2: 0, 6: 768}),
                              ([3, 7], {3: 0, 7: 640}),
                              ([4, 5], {4: 0, 5: 512})]
                    pts = [emit_group(*GROUPS[0]), emit_group(*GROUPS[1])]
                    emit_pv(*GROUPS[0], pts[0])
                    pts.append(emit_group(*GROUPS[2]))
                    emit_pv(*GROUPS[1], pts[1])
                    pts.append(emit_group(*GROUPS[3]))
                    emit_pv(*GROUPS[2], pts[2])
                    pts.append(emit_group(*GROUPS[4]))
                    emit_pv(*GROUPS[3], pts[3])
                    norm_half(0)       # acc0 complete: strips 4..7 are acc1-only
                    emit_pv(*GROUPS[4], pts[4])
                    evict_half(0)
                    norm_half(512)
                    evict_half(512)

                # prefetch next layer's weights HERE, on the same in-order
                # HWDGE queue as the softmax bounces: emitted after the head
                # loop, the transfers are forced to queue behind every
                # bounce (cross-queue issue order is NOT program order - the
                # idle Pool sequencer races ahead and its transfers would
                # otherwise block head 0's bounce on the serial DMA engine)
                if li + 1 < L:
                    wk_tiles[li + 1], wo_tiles[li + 1] = load_wk_wo(li + 1, nc.sync)
                    wff_tiles[li + 1] = load_wff(li + 1, nc.sync)
                # last-head bounce filler: two next-layer v-proj token tiles
                if li + 1 < L:
                    vproj(li + 1, wv_tiles[li + 1], range(0, 2))

                # ---- out projection + residual into x2_bf, ch-outer with
                # per-(ch, mt) psum pieces: chunk-0 LN1 stats overlap chunk-1
                # matmuls, hiding most of the LN chain
                ln1_st = new_stats()
                for ch in range(2):
                    c0 = ch * 512
                    cs = slice(c0, c0 + 512)
                    lp = new_lp()
                    for mt in range(NKT):
                        pf = psh.tile([128, 512], F32, tag="mmh", name="po")
                        for kt in range(NKT):
                            nc.tensor.matmul(pf,
                                             lhsT=wo[:, kt, mt * 128:mt * 128 + 128],
                                             rhs=outcat[:, kt, cs],
                                             start=(kt == 0), stop=(kt == NKT - 1))
                        nc.vector.scalar_tensor_tensor(
                            out=x2_bf[:, mt, cs], in0=pf,
                            scalar=params["bo"][:, li, mt:mt + 1],
                            in1=x_bf[:, mt, cs], op0=ALU.add, op1=ALU.add)
                        # square immediately: keeps the LN1 stats chain short
                        nc.scalar.activation(out=xsq_bf[:, mt, cs],
                                             in_=x2_bf[:, mt, cs], func=AF.Square)
                        # stats sums for tile mt-1, one evict behind
                        if mt >= 1:
                            ln_stats_mm(lp, mt - 1, ch)
                    ln_stats_mm(lp, NKT - 1, ch)
                    ln_stats_fin(lp, ch, *ln1_st)

                fil1 = ((lambda: vproj(li + 1, wv_tiles[li + 1], range(2, 5)))
                        if li + 1 < L else None)
                layernorm("l1s", "l1b", li, ln1_triv, filler=fil1, stats=ln1_st)
                # ffn1-head filler: vproj uses ps-pool psums (disjoint from
                # ffn1's psh pieces) and keeps the PE warm while the LN apply
                # + first h1 evictions drain on DVE/Act
                if li + 1 < L:
                    vproj(li + 1, wv_tiles[li + 1], range(5, 7))

                # ---- ffn1: h1 = relu(W1 @ x + b1), feature-major (bf16
                # matmuls). Evicted directly as an fp8 hi/lo pair for ffn2's
                # DoubleRow matmuls: hi = Act relu (fp8), lo = relu(psum) - hi
                # on DVE. Each (mt, ch) gets its own 1-bank psum piece from
                # psh: psum readers are chained in emission order, so small
                # pieces keep the relu->stt chain at 2 ops and release banks
                # early (the ps-pool [128,S] tiles would serialize 4 readers).
                for mt in range(NFT):
                    for ch in range(2):
                        c0 = ch * 512
                        pf = psh.tile([128, 512], F32, tag="mmh", name="pf")
                        for kt in range(NKT):
                            nc.tensor.matmul(pf,
                                             lhsT=w1[:, kt, mt * 128:mt * 128 + 128],
                                             rhs=x_bf[:, kt, c0:c0 + 512],
                                             start=(kt == 0), stop=(kt == NKT - 1))
                        nc.scalar.activation(out=h1_hi[ch][:, mt, :],
                                             in_=pf, func=AF.Relu,
                                             bias=params["b1"][:, li, mt:mt + 1])
                        nc.vector.scalar_tensor_tensor(
                            out=h1_lo[ch][:, mt, :], in0=pf, scalar=0.0,
                            in1=h1_hi[ch][:, mt, :], op0=ALU.max,
                            op1=ALU.subtract)

                # ---- ffn2 + residual into x2_bf, ch-outer with per-(ch, mt)
                # psum pieces. fp8 DoubleRow 3-term; psum = 2048*ff, descaled
                # in the eviction stt. Chunk-0 LN2 stats (and, on the final
                # layer, chunk-0 apply + output stores) overlap chunk-1.
                final = (li == L - 1)
                ln2_st = new_stats()
                for ch in range(2):
                    cs = slice(ch * 512, ch * 512 + 512)
                    lp = new_lp()
                    for mt in range(NKT):
                        pf = psh.tile([128, 512], F32, tag="mmh", name="pf2")
                        for half in range(2):
                            ci = half * 256
                            sl = pf[:, ci:ci + 256]
                            first = True
                            for ha, wa in ((h1_hi[ch], w2h), (h1_hi[ch], w2l),
                                           (h1_lo[ch], w2h)):
                                for ktp in range(0, NFT, 2):
                                    nc.tensor.matmul(
                                        sl,
                                        lhsT=wa[:, ktp:ktp + 2,
                                                mt * 128:mt * 128 + 128],
                                        rhs=ha[:, ktp:ktp + 2, ci:ci + 256],
                                        start=first,
                                        stop=(ha is h1_lo[ch] and ktp == NFT - 2),
                                        perf_mode=DR)
                                    first = False
                        nc.vector.scalar_tensor_tensor(
                            out=x2_bf[:, mt, cs], in0=pf,
                            scalar=1.0 / W2_SCL,
                            in1=x_bf[:, mt, cs], op0=ALU.mult, op1=ALU.add)
                        if not b2_zero:
                            nc.vector.tensor_scalar_add(
                                x2_bf[:, mt, cs], x2_bf[:, mt, cs],
                                params["b2"][:, li, mt:mt + 1])
                        # square immediately: keeps the LN2 stats chain short
                        nc.scalar.activation(out=xsq_bf[:, mt, cs],
                                             in_=x2_bf[:, mt, cs], func=AF.Square)
                        # stats sums for tile mt-1, one evict behind
                        if mt >= 1:
                            ln_stats_mm(lp, mt - 1, ch)
                    ln_stats_mm(lp, NKT - 1, ch)
                    ln_stats_fin(lp, ch, *ln2_st)
                    if final:
                        # stream this chunk straight out while the other
                        # chunk's ffn2 matmuls keep the PE busy
                        xout_ch = h1p.tile([128, NKT, 512], F32,
                                           tag=f"h1l{ch}", name=f"xout{ch}")
                        ln_apply_ch(ch, *ln2_st, "l2s", "l2b", li, ln2_triv,
                                    final=True, xout_ch=xout_ch)

                if not final:
                    fil2 = ((lambda: vproj(li + 1, wv_tiles[li + 1],
                                           range(7, NJT)))
                            if li + 1 < L else None)
                    layernorm("l2s", "l2b", li, ln2_triv, filler=fil2,
                              stats=ln2_st)
                    vx4 = _vx4_of[li + 1]

    # Pin every ACT instruction to the one table set that contains all the
    # functions this kernel uses (Exp/Ln/Identity/Relu/Square/Copy), so the
    # whole kernel needs a single ACT_TABLE_LOAD instead of thrashing between
    # the exp- and ln-anchored sets on every layernorm. Indices are preserved
    # (the pass emits act_func_set_id by list position).
    import concourse.bacc as _bacc_mod
    _orig_gat = _bacc_mod.get_activation_tables
    def _pinned_tables(arch):
        tabs = _orig_gat(arch)
        return {name: (funcs if name == "natural_log_exp_and_others" else set())
                for name, funcs in tabs.items()}
    _bacc_mod.get_activation_tables = _pinned_tables
    try:
        nc.compile()
    finally:
        _bacc_mod.get_activation_tables = _orig_gat
    return nc


def _pack_feat(arr, nt):
    """(L, nt*128) fp32 -> [128, L, nt]"""
    Ld = arr.shape[0]
    return np.ascontiguousarray(arr.reshape(Ld, nt, 128).transpose(2, 0, 1)).astype(np.float32)


def _split_f8(a, scale):
    """Split scale*a into fp8e4 hi + lo (hi = rne(x), lo = rne(x - hi))."""
    f8 = ml_dtypes.float8_e4m3
    x = np.clip(np.asarray(a, np.float32) * scale, -240.0, 240.0)
    hi = x.astype(f8)
    lo = np.clip(x - hi.astype(np.float32), -240.0, 240.0).astype(f8)
    return np.ascontiguousarray(hi), np.ascontiguousarray(lo)


def kernel(q_embed_data, qa_embed_data, pe, Wk, bk, Wv, bv, Wo, bo,
           ln1_s, ln1_b, W1, b1, W2, b2, ln2_s, ln2_b, **_unused):
    q = np.asarray(q_embed_data, np.float32)
    qa = np.asarray(qa_embed_data, np.float32)
    pe = np.asarray(pe, np.float32)
    bf = ml_dtypes.bfloat16

    has_bv = bool(np.any(np.asarray(bv)))
    bk_zero = not bool(np.any(np.asarray(bk)))
    b2_zero = not bool(np.any(np.asarray(b2)))
    ln1_triv = bool(np.all(np.asarray(ln1_s) == 1.0) and not np.any(np.asarray(ln1_b)))
    ln2_triv = bool(np.all(np.asarray(ln2_s) == 1.0) and not np.any(np.asarray(ln2_b)))
    key = (has_bv, bk_zero, ln1_triv, ln2_triv, b2_zero)
    if key not in _PROG_CACHE:
        _PROG_CACHE[key] = _build(has_bv, bk_zero, ln1_triv, ln2_triv, b2_zero)
    nc = _PROG_CACHE[key]

    wvT = np.asarray(Wv, np.float32).transpose(0, 2, 1)
    w2T = np.asarray(W2, np.float32).transpose(0, 2, 1)
    wvh, wvl = _split_f8(wvT, WV_SCL)
    w2h, w2l = _split_f8(w2T, W2_SCL)
    shared = {
        "wkT": np.ascontiguousarray(np.asarray(Wk, np.float32).transpose(0, 2, 1)).astype(bf),
        "wvTh": wvh, "wvTl": wvl,
        "woT": np.ascontiguousarray(np.asarray(Wo, np.float32).transpose(0, 2, 1)).astype(bf),
        "w1T": np.ascontiguousarray(np.asarray(W1, np.float32).transpose(0, 2, 1)).astype(bf),
        "w2Th": w2h, "w2Tl": w2l,
        "bkp": _pack_feat(np.asarray(bk, np.float32), NKT),
        "bop": _pack_feat(np.asarray(bo, np.float32), NKT),
        "b1p": _pack_feat(np.asarray(b1, np.float32), NFT),
        "b2p": _pack_feat(np.asarray(b2, np.float32), NKT),
        "l1s": _pack_feat(np.asarray(ln1_s, np.float32), NKT),
        "l1b": _pack_feat(np.asarray(ln1_b, np.float32), NKT),
        "l2s": _pack_feat(np.asarray(ln2_s, np.float32), NKT),
        "l2b": _pack_feat(np.asarray(ln2_b, np.float32), NKT),
        "mask01": (np.arange(128)[:, None] < np.arange(128)[None, :]).astype(bf),
    }
    if has_bv:
        shared["bvp"] = (np.asarray(bv, np.float32) * WV_SCL).reshape(1, L, D).astype(bf)

    in_maps = []
    for c in range(NCORES):
        m = dict(shared)
        m["xT"] = np.ascontiguousarray((q[c] + pe).T).astype(bf)
        yh, yl = _split_f8((qa[c] + pe).T, 1.0)
        m["yTh"] = yh
        m["yTl"] = yl
        in_maps.append(m)

    res = run_bass_kernel_spmd(nc, in_maps, core_ids=list(range(NCORES)))
    out = np.stack([np.ascontiguousarray(res.results[c]["outT"].T)
                    for c in range(NCORES)])
    return out.astype(np.float32)



# revision 49
# speedup vs baseline: 1.0595x; 1.0595x over previous
"""Trainium2 Bass kernel for a 4-layer dense transformer (AKT-style).

Sharding: data-parallel over batch. B=8 batch elements -> 1 per NeuronCore.
Each core runs the full 4-layer stack on its own (S=1024, D=512) slice with
no collectives; weights are replicated.

Per-core layout: feature-major activations [D, S] in bf16 (partition dim =
feature tiles of 128). The residual stream lives entirely in bf16 (x_bf =
post-LN, x2_bf = pre-LN residual); LN statistics come from PE ones-matmul
sums of the same bf16 tensors, so stats and apply are consistent. Matmuls
run in bf16 with fp32 PSUM accumulation. Attention uses the symmetric-scores
trick: S = kq @ kq^T is symmetric, so a [j, i]-layout strip of scores doubles
as the transposed-probabilities operand after a strictly-upper-triangular
causal mask; softmax denominators come from an extra ones-column appended to
V. The per-(head, i) normalizer is computed as a bf16 reciprocal straight
from the PSUM sums row, broadcast across 64 partitions (DRAM-bounce DMA for
heads 0..5, local K=1 PE matmul + SBUF copy for the last two heads where the
DMA round-trip would be exposed), and folded into the PSUM->SBUF eviction
multiply — TT ops may read at most one PSUM operand, so the broadcast side
lives in SBUF. Engine notes: sync=HWDGE carries startup loads, bounces, and
output stores in emission order (weight prefetch queues behind the bounces
on purpose); gpsimd=SWDGE only carries latency-tolerant params. Next-layer
v-projections are spread as PE filler under the softmax-exp (Act-bound) and
LN-stats chains.
"""
import sys

sys.path.insert(0, "/opt/trn_rl_repo")

import math

import ml_dtypes
import numpy as np

import concourse.bass as bass
import concourse.tile as tile
from concourse import bacc, mybir
from concourse.bass_utils import run_bass_kernel_spmd

F32 = mybir.dt.float32
BF16 = mybir.dt.bfloat16
F8 = mybir.dt.float8e4
DR = mybir.MatmulPerfMode.DoubleRow
AF = mybir.ActivationFunctionType
ALU = mybir.AluOpType

# fp8 split scales (powers of 2). Weights are scaled up so both the hi part
# and the hi-lo residual stay clear of the e4m3 subnormal floor; activations
# (x, y, h1) are O(1)..O(32) and need no scaling. psum descale happens at
# eviction.
WV_SCL = 32.0   # v-proj psum = 32 * v
W2_SCL = 64.0   # ffn2 psum = 64 * ff (h1 hi/lo is stored at natural scale)

B, S, D, H, FF, L = 8, 1024, 512, 8, 2048, 4
DK = D // H          # 64
NKT = D // 128       # 4  feature tiles
NJT = S // 128       # 8  token tiles
NFT = FF // 128      # 16 ffn tiles
SCALE = 1.0 / math.sqrt(DK)
EPS = 1e-5
NCORES = 8

_PROG_CACHE = {}


def _strip_chunks(a):
    """Column chunks (absolute i ranges) for scores/PV strip of j-tile a:
    the 128-wide diagonal block first, then pieces that cross neither an
    absolute 512-boundary (PV psum banks) nor a strip-local one (scores
    psum banks, local = absolute - 128*a)."""
    chunks = [(128 * a, 128 * a + 128)]
    start = 128 * a + 128
    pts = sorted({512, 128 * a + 512, S})
    for p in pts:
        if start < p <= S:
            chunks.append((start, p))
            start = p
    return chunks


def _build(has_bv, bk_zero=True, ln1_triv=True, ln2_triv=True, b2_zero=True):
    nc = bacc.Bacc("TRN2", target_bir_lowering=False, debug=False,
                   num_devices=NCORES)

    xT_e = nc.declare_dram_parameter("xT", [D, S], BF16, isOutput=False)
    yh_e = nc.declare_dram_parameter("yTh", [D, S], F8, isOutput=False)
    yl_e = nc.declare_dram_parameter("yTl", [D, S], F8, isOutput=False)
    wk_e = nc.declare_dram_parameter("wkT", [L, D, D], BF16, isOutput=False)
    wvh_e = nc.declare_dram_parameter("wvTh", [L, D, D], F8, isOutput=False)
    wvl_e = nc.declare_dram_parameter("wvTl", [L, D, D], F8, isOutput=False)
    wo_e = nc.declare_dram_parameter("woT", [L, D, D], BF16, isOutput=False)
    w1_e = nc.declare_dram_parameter("w1T", [L, D, FF], BF16, isOutput=False)
    w2h_e = nc.declare_dram_parameter("w2Th", [L, FF, D], F8, isOutput=False)
    w2l_e = nc.declare_dram_parameter("w2Tl", [L, FF, D], F8, isOutput=False)
    # per-feature params packed [128, L, ntiles]
    bk_e = nc.declare_dram_parameter("bkp", [128, L, NKT], F32, isOutput=False)
    bo_e = nc.declare_dram_parameter("bop", [128, L, NKT], F32, isOutput=False)
    b1_e = nc.declare_dram_parameter("b1p", [128, L, NFT], F32, isOutput=False)
    b2_e = nc.declare_dram_parameter("b2p", [128, L, NKT], F32, isOutput=False)
    l1s_e = nc.declare_dram_parameter("l1s", [128, L, NKT], F32, isOutput=False)
    l1b_e = nc.declare_dram_parameter("l1b", [128, L, NKT], F32, isOutput=False)
    l2s_e = nc.declare_dram_parameter("l2s", [128, L, NKT], F32, isOutput=False)
    l2b_e = nc.declare_dram_parameter("l2b", [128, L, NKT], F32, isOutput=False)
    bv_e = nc.declare_dram_parameter("bvp", [1, L, D], BF16, isOutput=False) if has_bv else None
    mask_e = nc.declare_dram_parameter("mask01", [128, 128], BF16, isOutput=False)
    out_e = nc.declare_dram_parameter("outT", [D, S], F32, isOutput=True)

    with tile.TileContext(nc) as tc:
        with (
            tc.tile_pool(name="res", bufs=1) as res,         # resident activations
            tc.tile_pool(name="wqkv", bufs=2) as wqkv,       # per-layer D x D weights
            tc.tile_pool(name="wff", bufs=2) as wff,         # per-layer ffn weights
            tc.tile_pool(name="pt", bufs=5) as ptp,          # exp'd prob strips
            tc.tile_pool(name="vp", bufs=2) as vp,           # v_ext double buffer
            tc.tile_pool(name="rc", bufs=3) as rcp,          # per-head recip rows
            tc.tile_pool(name="bc", bufs=1) as bc,           # LN broadcast tiles
            tc.tile_pool(name="tb", bufs=2) as tbp,
            tc.tile_pool(name="h1p", bufs=1) as h1p,
            tc.tile_pool(name="wff2", bufs=1) as wff2,          # LN apply temps
            tc.tile_pool(name="ps", bufs=2, space="PSUM") as ps,
            tc.tile_pool(name="psh", bufs=4, space="PSUM") as psh,
            tc.tile_pool(name="rb", bufs=3) as rbp,          # recip bcast rows
            tc.tile_pool(name="dr", bufs=4, space="DRAM") as dr,
        ):
            # ---- residents (bf16 residual stream)
            x_bf = res.tile([128, NKT, S], BF16, tag="x_bf")     # post-LN x
            x2_bf = res.tile([128, NKT, S], BF16, tag="x2_bf")   # pre-LN resid
            y_hi = res.tile([128, NKT, S], F8, tag="y_hi")
            y_lo = res.tile([128, NKT, S], F8, tag="y_lo")
            kq_bf = res.tile([128, NKT, S], BF16, tag="kq_bf")
            outcat = res.tile([128, NKT, S], BF16, tag="outcat")
            # h1 fp8 hi/lo, one tile per 512-token chunk: keeps the Act-relu
            # (hi) and DVE-stt (lo) evictions of different chunks free of
            # false subtile WARs (tracking is first-free-dim granular)
            h1_hi = [h1p.tile([128, NFT, 512], F8, tag=f"h1h{c}", name=f"h1h{c}")
                     for c in range(2)]
            h1_lo = [h1p.tile([128, NFT, 512], F8, tag=f"h1l{c}", name=f"h1l{c}")
                     for c in range(2)]
            xsq_bf = res.tile([128, NKT, S], BF16, tag="xsq")

            # ---- initial loads: x = q+pe, y = qa+pe precomputed on host.
            # sync = HWDGE (fast, low latency): carries the startup-critical
            # loads in need order (y+wv feed vproj first), then only the
            # latency-sensitive softmax bounces + output stores.
            # gpsimd = SWDGE (Q7 descgen ~1us, latency-tolerant): params,
            # mask, and all next-layer weight prefetches.
            yh4 = yh_e.rearrange("(k p) s -> p k s", p=128)
            yl4 = yl_e.rearrange("(k p) s -> p k s", p=128)

            def ln_stats_mm(lp, kt, ch):
                """One accumulation step of the per-chunk stats sums; emit
                kt lagged behind the producer's evict of feature tile kt so
                it never head-of-line-blocks the producer's own matmuls."""
                lp0, lp1 = lp
                cs = slice(ch * 512, ch * 512 + 512)
                nc.tensor.matmul(lp0, lhsT=ones128, rhs=x2_bf[:, kt, cs],
                                 start=(kt == 0), stop=(kt == NKT - 1))
                nc.tensor.matmul(lp1, lhsT=ones128, rhs=xsq_bf[:, kt, cs],
                                 start=(kt == 0), stop=(kt == NKT - 1))

            def new_lp():
                return (psh.tile([128, 512], F32, tag="mmh", name="lp0"),
                        psh.tile([128, 512], F32, tag="mmh", name="lp1"))

            def ln_stats_fin(lp, ch, meanb, sbv):
                """Finish one chunk's stats: mean/E[x2] eviction split across
                Act (mean, mean^2) and DVE (E[x2], var) so the serial chain is
                ~1us shorter than an all-DVE version; rstd via Ln/Exp."""
                lp0, lp1 = lp
                cs = slice(ch * 512, ch * 512 + 512)
                nc.scalar.mul(meanb[:, cs], lp0, 1.0 / D)
                nc.vector.tensor_scalar_mul(sbv[:, cs], lp1, 1.0 / D)
                m2 = tbp.tile([128, 512], BF16, tag="lntmp")
                nc.scalar.square(m2, meanb[:, cs])
                nc.vector.scalar_tensor_tensor(out=sbv[:, cs], in0=sbv[:, cs],
                                               scalar=float(EPS), in1=m2,
                                               op0=ALU.add, op1=ALU.subtract)
                nc.scalar.activation(out=sbv[:, cs], in_=sbv[:, cs], func=AF.Ln)
                nc.scalar.activation(out=sbv[:, cs], in_=sbv[:, cs],
                                     func=AF.Exp, scale=-0.5)

            def ln_stats_ch(ch, meanb, sbv):
                lp = new_lp()
                for kt in range(NKT):
                    ln_stats_mm(lp, kt, ch)
                ln_stats_fin(lp, ch, meanb, sbv)

            def new_stats():
                meanb = bc.tile([128, S], BF16, tag="meanb")
                sbv = bc.tile([128, S], BF16, tag="statb")  # ex2->var->rstd
                return meanb, sbv

            oute4 = out_e.rearrange("(k p) s -> p k s", p=128)

            def ln_apply_ch(ch, meanb, sbv, lname_s, lname_b, li, triv,
                            final=False, xout_ch=None):
                """Apply one chunk's LN: x = (x2 - mean) * rstd (+affine).
                Non-final+trivial uses 2 coarse DVE ops; final writes the
                fp32 staging tile and DMAs per kt."""
                cs = slice(ch * 512, ch * 512 + 512)
                if not final and triv:
                    # pairs of kt per op: x(kt0,kt1) lands early enough for
                    # the consumer matmuls' first accumulation steps while
                    # keeping the DVE queue short
                    for kt in range(0, NKT, 2):
                        mb = meanb[:, None, cs].to_broadcast([128, 2, 512])
                        sb = sbv[:, None, cs].to_broadcast([128, 2, 512])
                        nc.vector.tensor_sub(xsq_bf[:, kt:kt + 2, cs],
                                             x2_bf[:, kt:kt + 2, cs], mb)
                        nc.vector.tensor_mul(x_bf[:, kt:kt + 2, cs],
                                             xsq_bf[:, kt:kt + 2, cs], sb)
                    return
                for kt in range(NKT):
                    nc.vector.tensor_sub(xsq_bf[:, kt, cs],
                                         x2_bf[:, kt, cs], meanb[:, cs])
                    if final:
                        po = xout_ch[:, kt, :]
                        nc.vector.tensor_mul(po, xsq_bf[:, kt, cs], sbv[:, cs])
                        if not triv:
                            nc.vector.tensor_scalar(
                                out=po, in0=po,
                                scalar1=params[lname_s][:, li, kt:kt + 1],
                                scalar2=params[lname_b][:, li, kt:kt + 1],
                                op0=ALU.mult, op1=ALU.add)
                        nc.sync.dma_start(out=oute4[:, kt, cs], in_=po)
                    else:
                        nc.vector.tensor_mul(x_bf[:, kt, cs],
                                             xsq_bf[:, kt, cs], sbv[:, cs])
                        if not triv:
                            nc.vector.tensor_scalar(
                                out=x_bf[:, kt, cs], in0=x_bf[:, kt, cs],
                                scalar1=params[lname_s][:, li, kt:kt + 1],
                                scalar2=params[lname_b][:, li, kt:kt + 1],
                                op0=ALU.mult, op1=ALU.add)

            def layernorm(lname_s, lname_b, li, triv, filler=None, stats=None):
                """LN over features of x2_bf -> x_bf. Stats from PE
                ones-matmul sums of the same bf16 values the apply uses; pass
                `stats`=(meanb, sbv) when the producing phase already emitted
                per-ch stats."""
                if stats is None:
                    meanb, sbv = new_stats()
                    for ch in range(2):
                        ln_stats_ch(ch, meanb, sbv)
                else:
                    meanb, sbv = stats
                if filler is not None:
                    filler()
                for ch in range(2):
                    ln_apply_ch(ch, meanb, sbv, lname_s, lname_b, li, triv)


            def vproj(li, wv, jts):
                """v = y @ WvT (token-major) -- depends only on y hi/lo + wv,
                so it can fill PE bubbles in other phases. fp8 DoubleRow,
                3-term split: psum holds 32*v, descaled at eviction."""
                wvh, wvl = wv
                vx4 = _vx4_of[li]
                nc.gpsimd.memset(vx4[:, jts.start:jts.stop, :, 64:65], 1.0)
                for jt in jts:
                    pp = ps.tile([128, S], F32, tag="mm")
                    for half in range(2):
                        c0 = half * 256
                        sl = pp[:, c0:c0 + 256]
                        seq = [(y_hi, wvh, 0), (y_hi, wvh, 2),
                               (y_hi, wvl, 0), (y_hi, wvl, 2),
                               (y_lo, wvh, 0), (y_lo, wvh, 2)]
                        for i, (ya, wa, ktp) in enumerate(seq):
                            nc.tensor.matmul(
                                sl, lhsT=ya[:, ktp:ktp + 2, jt * 128:jt * 128 + 128],
                                rhs=wa[:, ktp:ktp + 2, c0:c0 + 256],
                                start=(i == 0),
                                stop=(i == len(seq) - 1) and not has_bv,
                                perf_mode=DR)
                        if has_bv:
                            nc.tensor.matmul(sl, lhsT=ones_row,
                                             rhs=bv_sb[0:1, li, c0:c0 + 256],
                                             start=False, stop=True,
                                             skip_group_check=True)
                    nc.scalar.mul(
                        vx4[:, jt, :, 0:64],
                        pp[:, 0:512].rearrange("p (h c) -> p h c", c=64),
                        1.0 / WV_SCL)
                return vx4

            _vx4_of = {}

            def new_vext(li):
                vext = vp.tile([128, NJT, H * 72], BF16, tag="vext")
                _vx4_of[li] = vext.rearrange("p j (h c) -> p j h c", c=72)
                return _vx4_of[li]

            wv_tiles = {}
            wk_tiles = {}
            wo_tiles = {}

            def load_wv(li, eng):
                th = wqkv.tile([128, NKT, D], F8, tag="wvh")
                eng.dma_start(out=th, in_=wvh_e[li].rearrange("(k p) m -> p k m", p=128))
                tl = wqkv.tile([128, NKT, D], F8, tag="wvl")
                eng.dma_start(out=tl, in_=wvl_e[li].rearrange("(k p) m -> p k m", p=128))
                return th, tl

            def load_wk_wo(li, eng):
                wk = wqkv.tile([128, NKT, D], BF16, tag="wk")
                eng.dma_start(out=wk, in_=wk_e[li].rearrange("(k p) m -> p k m", p=128))
                wo = wqkv.tile([128, NKT, D], BF16, tag="wo")
                eng.dma_start(out=wo, in_=wo_e[li].rearrange("(k p) m -> p k m", p=128))
                return wk, wo

            wff_tiles = {}

            def load_wff(li, eng):
                w1 = wff.tile([128, NKT, FF], BF16, tag="w1")
                eng.dma_start(out=w1, in_=w1_e[li].rearrange("(k p) m -> p k m", p=128))
                w2h = wff2.tile([128, NFT, D], F8, tag="w2h")
                eng.dma_start(out=w2h, in_=w2h_e[li].rearrange("(k p) m -> p k m", p=128))
                w2l = wff2.tile([128, NFT, D], F8, tag="w2l")
                eng.dma_start(out=w2l, in_=w2l_e[li].rearrange("(k p) m -> p k m", p=128))
                return w1, (w2h, w2l)

            nc.sync.dma_start(out=y_hi[:, :, 0:512], in_=yh4[:, :, 0:512])
            nc.sync.dma_start(out=y_lo[:, :, 0:512], in_=yl4[:, :, 0:512])
            wv_tiles[0] = load_wv(0, nc.sync)
            nc.sync.dma_start(out=y_hi[:, :, 512:1024], in_=yh4[:, :, 512:1024])
            nc.sync.dma_start(out=y_lo[:, :, 512:1024], in_=yl4[:, :, 512:1024])
            xT4 = xT_e.rearrange("(k p) s -> p k s", p=128)
            nc.sync.dma_start(out=x_bf[:, :, 0:512], in_=xT4[:, :, 0:512])
            nc.sync.dma_start(out=x_bf[:, :, 512:1024], in_=xT4[:, :, 512:1024])
            wk_tiles[0], wo_tiles[0] = load_wk_wo(0, nc.sync)
            wff_tiles[0] = load_wff(0, nc.sync)

            # ---- constants & params (issued after the critical-path loads)
            params = {}
            mask01 = res.tile([128, 128], BF16, tag="mask")
            nc.gpsimd.dma_start(out=mask01, in_=mask_e[:])
            ones128 = res.tile([128, 128], BF16, tag="ones")
            nc.vector.memset(ones128, 1.0)
            ones_1x64 = res.tile([1, 64], BF16, tag="ones64")
            nc.vector.memset(ones_1x64, 1.0)
            if has_bv:
                ones_row = res.tile([1, 128], BF16, tag="onesr")
                nc.vector.memset(ones_row, 1.0)
                bv_sb = res.tile([1, L, D], BF16, tag="bv")
                nc.gpsimd.dma_start(out=bv_sb, in_=bv_e[:])
            for name, ext, nt in (("bk", bk_e, NKT), ("bo", bo_e, NKT),
                                  ("b1", b1_e, NFT), ("b2", b2_e, NKT),
                                  ("l1s", l1s_e, NKT), ("l1b", l1b_e, NKT),
                                  ("l2s", l2s_e, NKT), ("l2b", l2b_e, NKT)):
                t = res.tile([128, L, nt], F32, tag="prm_" + name)
                nc.gpsimd.dma_start(out=t, in_=ext[:])
                params[name] = t

            new_vext(0)
            vx4 = vproj(0, wv_tiles[0], range(0, NJT))

            for li in range(L):
                wk, wo = wk_tiles[li], wo_tiles[li]
                w1, (w2h, w2l) = wff_tiles[li]
                if li + 1 < L:
                    new_vext(li + 1)
                    # wv early on the same HWDGE queue: its transfer lands
                    # during kq-proj, before the first softmax bounce
                    wv_tiles[li + 1] = load_wv(li + 1, nc.sync)

                # ---- kq projection tile mt feeds heads 2mt, 2mt+1:
                # interleave so exps start after one kq tile and later kq
                # tiles keep the PE busy under the early heads' exps
                def kq_tile(mt):
                    pp = ps.tile([128, S], F32, tag="mm")
                    for ch in range(2):
                        c0 = ch * 512
                        for kt in range(NKT):
                            nc.tensor.matmul(pp[:, c0:c0 + 512],
                                             lhsT=wk[:, kt, mt * 128:mt * 128 + 128],
                                             rhs=x_bf[:, kt, c0:c0 + 512],
                                             start=(kt == 0), stop=(kt == NKT - 1))
                    # DVE eviction: keeps the Act queue clear so head-0 exps
                    # aren't stuck behind three unrelated kq evictions
                    if bk_zero:
                        nc.vector.tensor_copy(kq_bf[:, mt, :], pp)
                    else:
                        nc.vector.tensor_scalar_add(
                            kq_bf[:, mt, :], pp, params["bk"][:, li, mt:mt + 1])

                for mt in range(NKT):
                    kq_tile(mt)

                # ---- attention, head by head
                for h in range(H):
                    po = (h % 2) * 64
                    kqh = kq_bf[po:po + 64, h // 2, :]
                    acc0 = psh.tile([128, 512], F32, tag="mmh")
                    acc1 = psh.tile([128, 512], F32, tag="mmh")
                    # strip groups share one psum tile + one exp: strips
                    # 4..7 are narrow enough to pack pairwise with no padding
                    # (local offsets chosen so no matmul crosses a psum bank)
                    def emit_group(group, locs):
                        sc = ps.tile([128, S], F32, tag="mm")
                        for a in group:
                            for (s0, e0) in _strip_chunks(a):
                                lo = locs[a] + s0 - 128 * a
                                nc.tensor.matmul(sc[:, lo:lo + (e0 - s0)],
                                                 lhsT=kqh[:, 128 * a:128 * a + 128],
                                                 rhs=kqh[:, s0:s0 + (e0 - s0)],
                                                 start=True, stop=True)
                        wtot = max(locs[a] + S - 128 * a for a in group)
                        pt = ptp.tile([128, S], BF16, tag="pt")
                        nc.scalar.activation(out=pt[:, 0:wtot], in_=sc[:, 0:wtot],
                                             func=AF.Exp, scale=float(SCALE))
                        if (len(group) == 2
                                and locs[group[1]] - locs[group[0]] <= 512):
                            stride = locs[group[1]] - locs[group[0]]
                            view = pt[:, 0:stride * len(group)].rearrange(
                                "p (g c) -> p g c", c=stride)
                            nc.vector.tensor_mul(view[:, :, 0:128], view[:, :, 0:128],
                                                 mask01[:, None, :].to_broadcast(
                                                     [128, len(group), 128]))
                        else:
                            for a in group:
                                lo = locs[a]
                                nc.vector.tensor_mul(pt[:, lo:lo + 128],
                                                     pt[:, lo:lo + 128], mask01)
                        return pt

                    def emit_pv(group, locs, pt):
                        # PV needs only the diagonal + absolute-512 splits
                        # (strip-local splits are a scores-psum constraint)
                        for a in group:
                            chunks = [(128 * a, 128 * a + 128)]
                            st = 128 * a + 128
                            for p in (512, S):
                                if st < p <= S:
                                    chunks.append((st, p))
                                    st = p
                            for (s0, e0) in chunks:
                                acc = acc0 if s0 < 512 else acc1
                                o0 = s0 - (0 if s0 < 512 else 512)
                                lo = locs[a] + s0 - 128 * a
                                nc.tensor.matmul(acc[0:65, o0:o0 + (e0 - s0)],
                                                 lhsT=vx4[:, a, h, 0:65],
                                                 rhs=pt[:, lo:lo + (e0 - s0)],
                                                 start=(a == 0),
                                                 stop=(e0 == 128 * a + 128),
                                                 skip_group_check=True)

                    # softmax denominators: bf16 reciprocal straight from the
                    # psum ones-row, col 0 zeroed (empty first query row), then
                    # a DRAM-bounce DMA (HWDGE, low latency) broadcasts it
                    # across 64 partitions and the eviction multiply
                    # normalizes straight out of psum (TT ops may read at most
                    # ONE operand from PSUM, so the broadcast side must be
                    # SBUF). acc0 (cols 0..512) finishes one PV group before
                    # acc1, so its bounce overlaps the last PV group.
                    recip = rcp.tile([1, S], BF16, tag="recip")
                    bnc = dr.tile([1, S], BF16, tag="bounce")
                    rbh = rbp.tile([64, S], BF16, tag="rbh")

                    def norm_half(c0):
                        acc = acc0 if c0 == 0 else acc1
                        cs = slice(c0, c0 + 512)
                        with nc.allow_low_precision(reason="softmax 1/l bf16"):
                            nc.vector.reciprocal(recip[:, cs], acc[64:65, :])
                        if c0 == 0:
                            nc.gpsimd.memset(recip[:, 0:1], 0.0)
                        if h >= H - 2:
                            # last head: local broadcast (PE matmul -> psum
                            # -> SBUF copy) instead of the DRAM bounce; the
                            # ~5us DMA round-trip would be fully exposed at
                            # the phase boundary
                            bps = psh.tile([128, 512], F32, tag="mmh")
                            nc.tensor.matmul(bps[0:64, :], lhsT=ones_1x64,
                                             rhs=recip[0:1, cs],
                                             start=True, stop=True)
                            nc.vector.tensor_copy(rbh[:, cs], bps[0:64, :])
                        else:
                            nc.sync.dma_start(out=bnc[:, cs], in_=recip[:, cs])
                            bap = bnc[:, cs]
                            nc.sync.dma_start(out=rbh[:, cs], in_=bass.AP(
                                tensor=bap.tensor, offset=bap.offset,
                                ap=[[0, 64]] + bap.ap[1:]))

                    def evict_half(c0):
                        acc = acc0 if c0 == 0 else acc1
                        cs = slice(c0, c0 + 512)
                        nc.vector.tensor_mul(outcat[po:po + 64, h // 2, cs],
                                             acc[0:64, :], rbh[:, cs])

                    GROUPS = [([0], {0: 0}), ([1], {1: 0}),
                              ([2, 6], {2: 0, 6: 768}),
                              ([3, 7], {3: 0, 7: 640}),
                              ([4, 5], {4: 0, 5: 512})]
                    pts = [emit_group(*GROUPS[0]), emit_group(*GROUPS[1])]
                    emit_pv(*GROUPS[0], pts[0])
                    pts.append(emit_group(*GROUPS[2]))
                    emit_pv(*GROUPS[1], pts[1])
                    pts.append(emit_group(*GROUPS[3]))
                    emit_pv(*GROUPS[2], pts[2])
                    pts.append(emit_group(*GROUPS[4]))
                    emit_pv(*GROUPS[3], pts[3])
                    norm_half(0)       # acc0 complete: strips 4..7 are acc1-only
                    emit_pv(*GROUPS[4], pts[4])
                    evict_half(0)
                    norm_half(512)
                    evict_half(512)

                # prefetch next layer's weights HERE, on the same in-order
                # HWDGE queue as the softmax bounces: emitted after the head
                # loop, the transfers are forced to queue behind every
                # bounce (cross-queue issue order is NOT program order - the
                # idle Pool sequencer races ahead and its transfers would
                # otherwise block head 0's bounce on the serial DMA engine)
                if li + 1 < L:
                    wk_tiles[li + 1], wo_tiles[li + 1] = load_wk_wo(li + 1, nc.sync)
                    wff_tiles[li + 1] = load_wff(li + 1, nc.sync)
                # last-head bounce filler: two next-layer v-proj token tiles
                if li + 1 < L:
                    vproj(li + 1, wv_tiles[li + 1], range(0, 2))

                # ---- out projection + residual into x2_bf, ch-outer with
                # per-(ch, mt) psum pieces: chunk-0 LN1 stats overlap chunk-1
                # matmuls, hiding most of the LN chain
                ln1_st = new_stats()
                for ch in range(2):
                    c0 = ch * 512
                    cs = slice(c0, c0 + 512)
                    lp = new_lp()
                    for mt in range(NKT):
                        pf = psh.tile([128, 512], F32, tag="mmh", name="po")
                        for kt in range(NKT):
                            nc.tensor.matmul(pf,
                                             lhsT=wo[:, kt, mt * 128:mt * 128 + 128],
                                             rhs=outcat[:, kt, cs],
                                             start=(kt == 0), stop=(kt == NKT - 1))
                        nc.vector.scalar_tensor_tensor(
                            out=x2_bf[:, mt, cs], in0=pf,
                            scalar=params["bo"][:, li, mt:mt + 1],
                            in1=x_bf[:, mt, cs], op0=ALU.add, op1=ALU.add)
                        # square immediately: keeps the LN1 stats chain short
                        nc.scalar.activation(out=xsq_bf[:, mt, cs],
                                             in_=x2_bf[:, mt, cs], func=AF.Square)
                        # stats sums lag two evicts behind so the PE never
                        # head-of-line-waits on the Act square of that tile
                        if mt >= 2:
                            ln_stats_mm(lp, mt - 2, ch)
                    ln_stats_mm(lp, NKT - 2, ch)
                    ln_stats_mm(lp, NKT - 1, ch)
                    ln_stats_fin(lp, ch, *ln1_st)

                fil1 = ((lambda: vproj(li + 1, wv_tiles[li + 1], range(2, 5)))
                        if li + 1 < L else None)
                layernorm("l1s", "l1b", li, ln1_triv, filler=fil1, stats=ln1_st)
                # ffn1-head filler: vproj uses ps-pool psums (disjoint from
                # ffn1's psh pieces) and keeps the PE warm while the LN apply
                # + first h1 evictions drain on DVE/Act
                if li + 1 < L:
                    vproj(li + 1, wv_tiles[li + 1], range(5, 7))

                # ---- ffn1: h1 = relu(W1 @ x + b1), feature-major (bf16
                # matmuls). Evicted directly as an fp8 hi/lo pair for ffn2's
                # DoubleRow matmuls: hi = Act relu (fp8), lo = relu(psum) - hi
                # on DVE. Each (mt, ch) gets its own 1-bank psum piece from
                # psh: psum readers are chained in emission order, so small
                # pieces keep the relu->stt chain at 2 ops and release banks
                # early (the ps-pool [128,S] tiles would serialize 4 readers).
                for mt in range(NFT):
                    for ch in range(2):
                        c0 = ch * 512
                        pf = psh.tile([128, 512], F32, tag="mmh", name="pf")
                        for kt in range(NKT):
                            nc.tensor.matmul(pf,
                                             lhsT=w1[:, kt, mt * 128:mt * 128 + 128],
                                             rhs=x_bf[:, kt, c0:c0 + 512],
                                             start=(kt == 0), stop=(kt == NKT - 1))
                        nc.scalar.activation(out=h1_hi[ch][:, mt, :],
                                             in_=pf, func=AF.Relu,
                                             bias=params["b1"][:, li, mt:mt + 1])
                        nc.vector.scalar_tensor_tensor(
                            out=h1_lo[ch][:, mt, :], in0=pf, scalar=0.0,
                            in1=h1_hi[ch][:, mt, :], op0=ALU.max,
                            op1=ALU.subtract)

                # ---- ffn2 + residual into x2_bf, ch-outer with per-(ch, mt)
                # psum pieces. fp8 DoubleRow 3-term; psum = 2048*ff, descaled
                # in the eviction stt. Chunk-0 LN2 stats (and, on the final
                # layer, chunk-0 apply + output stores) overlap chunk-1.
                final = (li == L - 1)
                ln2_st = new_stats()
                for ch in range(2):
                    cs = slice(ch * 512, ch * 512 + 512)
                    lp = new_lp()
                    for mt in range(NKT):
                        pf = psh.tile([128, 512], F32, tag="mmh", name="pf2")
                        for half in range(2):
                            ci = half * 256
                            sl = pf[:, ci:ci + 256]
                            first = True
                            for ha, wa in ((h1_hi[ch], w2h), (h1_hi[ch], w2l),
                                           (h1_lo[ch], w2h)):
                                for ktp in range(0, NFT, 2):
                                    nc.tensor.matmul(
                                        sl,
                                        lhsT=wa[:, ktp:ktp + 2,
                                                mt * 128:mt * 128 + 128],
                                        rhs=ha[:, ktp:ktp + 2, ci:ci + 256],
                                        start=first,
                                        stop=(ha is h1_lo[ch] and ktp == NFT - 2),
                                        perf_mode=DR)
                                    first = False
                        nc.vector.scalar_tensor_tensor(
                            out=x2_bf[:, mt, cs], in0=pf,
                            scalar=1.0 / W2_SCL,
                            in1=x_bf[:, mt, cs], op0=ALU.mult, op1=ALU.add)
                        if not b2_zero:
                            nc.vector.tensor_scalar_add(
                                x2_bf[:, mt, cs], x2_bf[:, mt, cs],
                                params["b2"][:, li, mt:mt + 1])
                        # square immediately: keeps the LN2 stats chain short
                        nc.scalar.activation(out=xsq_bf[:, mt, cs],
                                             in_=x2_bf[:, mt, cs], func=AF.Square)
                        # stats sums lag two evicts behind so the PE never
                        # head-of-line-waits on the Act square of that tile
                        if mt >= 2:
                            ln_stats_mm(lp, mt - 2, ch)
                    ln_stats_mm(lp, NKT - 2, ch)
                    ln_stats_mm(lp, NKT - 1, ch)
                    ln_stats_fin(lp, ch, *ln2_st)
                    if final:
                        # stream this chunk straight out while the other
                        # chunk's ffn2 matmuls keep the PE busy
                        xout_ch = h1p.tile([128, NKT, 512], F32,
                                           tag=f"h1l{ch}", name=f"xout{ch}")
                        ln_apply_ch(ch, *ln2_st, "l2s", "l2b", li, ln2_triv,
                                    final=True, xout_ch=xout_ch)

                if not final:
                    fil2 = ((lambda: vproj(li + 1, wv_tiles[li + 1],
                                           range(7, NJT)))
                            if li + 1 < L else None)
                    layernorm("l2s", "l2b", li, ln2_triv, filler=fil2,
                              stats=ln2_st)
                    vx4 = _vx4_of[li + 1]

    # Pin every ACT instruction to the one table set that contains all the
    # functions this kernel uses (Exp/Ln/Identity/Relu/Square/Copy), so the
    # whole kernel needs a single ACT_TABLE_LOAD instead of thrashing between
    # the exp- and ln-anchored sets on every layernorm. Indices are preserved
    # (the pass emits act_func_set_id by list position).
    import concourse.bacc as _bacc_mod
    _orig_gat = _bacc_mod.get_activation_tables
    def _pinned_tables(arch):
        tabs = _orig_gat(arch)
        return {name: (funcs if name == "natural_log_exp_and_others" else set())
                for name, funcs in tabs.items()}
    _bacc_mod.get_activation_tables = _pinned_tables
    try:
        nc.compile()
    finally:
        _bacc_mod.get_activation_tables = _orig_gat
    return nc


def _pack_feat(arr, nt):
    """(L, nt*128) fp32 -> [128, L, nt]"""
    Ld = arr.shape[0]
    return np.ascontiguousarray(arr.reshape(Ld, nt, 128).transpose(2, 0, 1)).astype(np.float32)


def _split_f8(a, scale):
    """Split scale*a into fp8e4 hi + lo (hi = rne(x), lo = rne(x - hi))."""
    f8 = ml_dtypes.float8_e4m3
    x = np.clip(np.asarray(a, np.float32) * scale, -240.0, 240.0)
    hi = x.astype(f8)
    lo = np.clip(x - hi.astype(np.float32), -240.0, 240.0).astype(f8)
    return np.ascontiguousarray(hi), np.ascontiguousarray(lo)


def kernel(q_embed_data, qa_embed_data, pe, Wk, bk, Wv, bv, Wo, bo,
           ln1_s, ln1_b, W1, b1, W2, b2, ln2_s, ln2_b, **_unused):
    q = np.asarray(q_embed_data, np.float32)
    qa = np.asarray(qa_embed_data, np.float32)
    pe = np.asarray(pe, np.float32)
    bf = ml_dtypes.bfloat16

    has_bv = bool(np.any(np.asarray(bv)))
    bk_zero = not bool(np.any(np.asarray(bk)))
    b2_zero = not bool(np.any(np.asarray(b2)))
    ln1_triv = bool(np.all(np.asarray(ln1_s) == 1.0) and not np.any(np.asarray(ln1_b)))
    ln2_triv = bool(np.all(np.asarray(ln2_s) == 1.0) and not np.any(np.asarray(ln2_b)))
    key = (has_bv, bk_zero, ln1_triv, ln2_triv, b2_zero)
    if key not in _PROG_CACHE:
        _PROG_CACHE[key] = _build(has_bv, bk_zero, ln1_triv, ln2_triv, b2_zero)
    nc = _PROG_CACHE[key]

    wvT = np.asarray(Wv, np.float32).transpose(0, 2, 1)
    w2T = np.asarray(W2, np.float32).transpose(0, 2, 1)
    wvh, wvl = _split_f8(wvT, WV_SCL)
    w2h, w2l = _split_f8(w2T, W2_SCL)
    shared = {
        "wkT": np.ascontiguousarray(np.asarray(Wk, np.float32).transpose(0, 2, 1)).astype(bf),
        "wvTh": wvh, "wvTl": wvl,
        "woT": np.ascontiguousarray(np.asarray(Wo, np.float32).transpose(0, 2, 1)).astype(bf),
        "w1T": np.ascontiguousarray(np.asarray(W1, np.float32).transpose(0, 2, 1)).astype(bf),
        "w2Th": w2h, "w2Tl": w2l,
        "bkp": _pack_feat(np.asarray(bk, np.float32), NKT),
        "bop": _pack_feat(np.asarray(bo, np.float32), NKT),
        "b1p": _pack_feat(np.asarray(b1, np.float32), NFT),
        "b2p": _pack_feat(np.asarray(b2, np.float32), NKT),
        "l1s": _pack_feat(np.asarray(ln1_s, np.float32), NKT),
        "l1b": _pack_feat(np.asarray(ln1_b, np.float32), NKT),
        "l2s": _pack_feat(np.asarray(ln2_s, np.float32), NKT),
        "l2b": _pack_feat(np.asarray(ln2_b, np.float32), NKT),
        "mask01": (np.arange(128)[:, None] < np.arange(128)[None, :]).astype(bf),
    }
    if has_bv:
        shared["bvp"] = (np.asarray(bv, np.float32) * WV_SCL).reshape(1, L, D).astype(bf)

    in_maps = []
    for c in range(NCORES):
        m = dict(shared)
        m["xT"] = np.ascontiguousarray((q[c] + pe).T).astype(bf)
        yh, yl = _split_f8((qa[c] + pe).T, 1.0)
        m["yTh"] = yh
        m["yTl"] = yl
        in_maps.append(m)

    res = run_bass_kernel_spmd(nc, in_maps, core_ids=list(range(NCORES)))
    out = np.stack([np.ascontiguousarray(res.results[c]["outT"].T)
                    for c in range(NCORES)])
    return out.astype(np.float32)



# revision 50
# speedup vs baseline: 1.0653x; 1.0055x over previous
"""Trainium2 Bass kernel for a 4-layer dense transformer (AKT-style).

Sharding: data-parallel over batch. B=8 batch elements -> 1 per NeuronCore.
Each core runs the full 4-layer stack on its own (S=1024, D=512) slice with
no collectives; weights are replicated.

Per-core layout: feature-major activations [D, S] in bf16 (partition dim =
feature tiles of 128). The residual stream lives entirely in bf16 (x_bf =
post-LN, x2_bf = pre-LN residual); LN statistics come from PE ones-matmul
sums of the same bf16 tensors, so stats and apply are consistent. Matmuls
run in bf16 with fp32 PSUM accumulation. Attention uses the symmetric-scores
trick: S = kq @ kq^T is symmetric, so a [j, i]-layout strip of scores doubles
as the transposed-probabilities operand after a strictly-upper-triangular
causal mask; softmax denominators come from an extra ones-column appended to
V. The per-(head, i) normalizer is computed as a bf16 reciprocal straight
from the PSUM sums row, broadcast across 64 partitions (DRAM-bounce DMA for
heads 0..5, local K=1 PE matmul + SBUF copy for the last two heads where the
DMA round-trip would be exposed), and folded into the PSUM->SBUF eviction
multiply — TT ops may read at most one PSUM operand, so the broadcast side
lives in SBUF. Engine notes: sync=HWDGE carries startup loads, bounces, and
output stores in emission order (weight prefetch queues behind the bounces
on purpose); gpsimd=SWDGE only carries latency-tolerant params. Next-layer
v-projections are spread as PE filler under the softmax-exp (Act-bound) and
LN-stats chains.
"""
import sys

sys.path.insert(0, "/opt/trn_rl_repo")

import math

import ml_dtypes
import numpy as np

import concourse.bass as bass
import concourse.tile as tile
from concourse import bacc, mybir
from concourse.bass_utils import run_bass_kernel_spmd

F32 = mybir.dt.float32
BF16 = mybir.dt.bfloat16
F8 = mybir.dt.float8e4
DR = mybir.MatmulPerfMode.DoubleRow
AF = mybir.ActivationFunctionType
ALU = mybir.AluOpType

# fp8 split scales (powers of 2). Weights are scaled up so both the hi part
# and the hi-lo residual stay clear of the e4m3 subnormal floor; activations
# (x, y, h1) are O(1)..O(32) and need no scaling. psum descale happens at
# eviction.
WV_SCL = 32.0   # v-proj psum = 32 * v
W2_SCL = 64.0   # ffn2 psum = 64 * ff (h1 hi/lo is stored at natural scale)

B, S, D, H, FF, L = 8, 1024, 512, 8, 2048, 4
DK = D // H          # 64
NKT = D // 128       # 4  feature tiles
NJT = S // 128       # 8  token tiles
NFT = FF // 128      # 16 ffn tiles
SCALE = 1.0 / math.sqrt(DK)
EPS = 1e-5
NCORES = 8

_PROG_CACHE = {}


def _strip_chunks(a):
    """Column chunks (absolute i ranges) for scores/PV strip of j-tile a:
    the 128-wide diagonal block first, then pieces that cross neither an
    absolute 512-boundary (PV psum banks) nor a strip-local one (scores
    psum banks, local = absolute - 128*a)."""
    chunks = [(128 * a, 128 * a + 128)]
    start = 128 * a + 128
    pts = sorted({512, 128 * a + 512, S})
    for p in pts:
        if start < p <= S:
            chunks.append((start, p))
            start = p
    return chunks


def _build(has_bv, bk_zero=True, ln1_triv=True, ln2_triv=True, b2_zero=True):
    nc = bacc.Bacc("TRN2", target_bir_lowering=False, debug=False,
                   num_devices=NCORES)

    xT_e = nc.declare_dram_parameter("xT", [D, S], BF16, isOutput=False)
    yh_e = nc.declare_dram_parameter("yTh", [D, S], F8, isOutput=False)
    yl_e = nc.declare_dram_parameter("yTl", [D, S], F8, isOutput=False)
    wk_e = nc.declare_dram_parameter("wkT", [L, D, D], BF16, isOutput=False)
    wvh_e = nc.declare_dram_parameter("wvTh", [L, D, D], F8, isOutput=False)
    wvl_e = nc.declare_dram_parameter("wvTl", [L, D, D], F8, isOutput=False)
    wo_e = nc.declare_dram_parameter("woT", [L, D, D], BF16, isOutput=False)
    w1_e = nc.declare_dram_parameter("w1T", [L, D, FF], BF16, isOutput=False)
    w2h_e = nc.declare_dram_parameter("w2Th", [L, FF, D], F8, isOutput=False)
    w2l_e = nc.declare_dram_parameter("w2Tl", [L, FF, D], F8, isOutput=False)
    # per-feature params packed [128, L, ntiles]
    bk_e = nc.declare_dram_parameter("bkp", [128, L, NKT], F32, isOutput=False)
    bo_e = nc.declare_dram_parameter("bop", [128, L, NKT], F32, isOutput=False)
    b1_e = nc.declare_dram_parameter("b1p", [128, L, NFT], F32, isOutput=False)
    b2_e = nc.declare_dram_parameter("b2p", [128, L, NKT], F32, isOutput=False)
    l1s_e = nc.declare_dram_parameter("l1s", [128, L, NKT], F32, isOutput=False)
    l1b_e = nc.declare_dram_parameter("l1b", [128, L, NKT], F32, isOutput=False)
    l2s_e = nc.declare_dram_parameter("l2s", [128, L, NKT], F32, isOutput=False)
    l2b_e = nc.declare_dram_parameter("l2b", [128, L, NKT], F32, isOutput=False)
    bv_e = nc.declare_dram_parameter("bvp", [1, L, D], BF16, isOutput=False) if has_bv else None
    mask_e = nc.declare_dram_parameter("mask01", [128, 128], BF16, isOutput=False)
    out_e = nc.declare_dram_parameter("outT", [D, S], F32, isOutput=True)

    with tile.TileContext(nc) as tc:
        with (
            tc.tile_pool(name="res", bufs=1) as res,         # resident activations
            tc.tile_pool(name="wqkv", bufs=2) as wqkv,       # per-layer D x D weights
            tc.tile_pool(name="wff", bufs=2) as wff,         # per-layer ffn weights
            tc.tile_pool(name="pt", bufs=5) as ptp,          # exp'd prob strips
            tc.tile_pool(name="vp", bufs=2) as vp,           # v_ext double buffer
            tc.tile_pool(name="rc", bufs=3) as rcp,          # per-head recip rows
            tc.tile_pool(name="bc", bufs=1) as bc,           # LN broadcast tiles
            tc.tile_pool(name="tb", bufs=2) as tbp,
            tc.tile_pool(name="h1p", bufs=1) as h1p,
            tc.tile_pool(name="wff2", bufs=1) as wff2,          # LN apply temps
            tc.tile_pool(name="ps", bufs=2, space="PSUM") as ps,
            tc.tile_pool(name="psh", bufs=4, space="PSUM") as psh,
            tc.tile_pool(name="rb", bufs=3) as rbp,          # recip bcast rows
            tc.tile_pool(name="dr", bufs=4, space="DRAM") as dr,
        ):
            # ---- residents (bf16 residual stream)
            x_bf = res.tile([128, NKT, S], BF16, tag="x_bf")     # post-LN x
            x2_bf = res.tile([128, NKT, S], BF16, tag="x2_bf")   # pre-LN resid
            y_hi = res.tile([128, NKT, S], F8, tag="y_hi")
            y_lo = res.tile([128, NKT, S], F8, tag="y_lo")
            kq_bf = res.tile([128, NKT, S], BF16, tag="kq_bf")
            outcat = res.tile([128, NKT, S], BF16, tag="outcat")
            # h1 fp8 hi/lo, one tile per 512-token chunk: keeps the Act-relu
            # (hi) and DVE-stt (lo) evictions of different chunks free of
            # false subtile WARs (tracking is first-free-dim granular)
            h1_hi = [h1p.tile([128, NFT, 512], F8, tag=f"h1h{c}", name=f"h1h{c}")
                     for c in range(2)]
            h1_lo = [h1p.tile([128, NFT, 512], F8, tag=f"h1l{c}", name=f"h1l{c}")
                     for c in range(2)]
            xsq_bf = res.tile([128, NKT, S], BF16, tag="xsq")

            # ---- initial loads: x = q+pe, y = qa+pe precomputed on host.
            # sync = HWDGE (fast, low latency): carries the startup-critical
            # loads in need order (y+wv feed vproj first), then only the
            # latency-sensitive softmax bounces + output stores.
            # gpsimd = SWDGE (Q7 descgen ~1us, latency-tolerant): params,
            # mask, and all next-layer weight prefetches.
            yh4 = yh_e.rearrange("(k p) s -> p k s", p=128)
            yl4 = yl_e.rearrange("(k p) s -> p k s", p=128)

            def ln_stats_mm(lp, kt, ch):
                """One accumulation step of the per-chunk stats sums; emit
                kt lagged behind the producer's evict of feature tile kt so
                it never head-of-line-blocks the producer's own matmuls."""
                lp0, lp1 = lp
                cs = slice(ch * 512, ch * 512 + 512)
                nc.tensor.matmul(lp0, lhsT=ones128, rhs=x2_bf[:, kt, cs],
                                 start=(kt == 0), stop=(kt == NKT - 1))
                nc.tensor.matmul(lp1, lhsT=ones128, rhs=xsq_bf[:, kt, cs],
                                 start=(kt == 0), stop=(kt == NKT - 1))

            def new_lp():
                return (psh.tile([128, 512], F32, tag="mmh", name="lp0"),
                        psh.tile([128, 512], F32, tag="mmh", name="lp1"))

            def ln_stats_fin(lp, ch, meanb, sbv):
                """Finish one chunk's stats: mean/E[x2] eviction split across
                Act (mean, mean^2) and DVE (E[x2], var) so the serial chain is
                ~1us shorter than an all-DVE version; rstd via Ln/Exp."""
                lp0, lp1 = lp
                cs = slice(ch * 512, ch * 512 + 512)
                nc.scalar.mul(meanb[:, cs], lp0, 1.0 / D)
                nc.vector.tensor_scalar_mul(sbv[:, cs], lp1, 1.0 / D)
                m2 = tbp.tile([128, 512], BF16, tag="lntmp")
                nc.scalar.square(m2, meanb[:, cs])
                nc.vector.scalar_tensor_tensor(out=sbv[:, cs], in0=sbv[:, cs],
                                               scalar=float(EPS), in1=m2,
                                               op0=ALU.add, op1=ALU.subtract)
                nc.scalar.activation(out=sbv[:, cs], in_=sbv[:, cs], func=AF.Ln)
                nc.scalar.activation(out=sbv[:, cs], in_=sbv[:, cs],
                                     func=AF.Exp, scale=-0.5)

            def ln_stats_ch(ch, meanb, sbv):
                lp = new_lp()
                for kt in range(NKT):
                    ln_stats_mm(lp, kt, ch)
                ln_stats_fin(lp, ch, meanb, sbv)

            def new_stats():
                meanb = bc.tile([128, S], BF16, tag="meanb")
                sbv = bc.tile([128, S], BF16, tag="statb")  # ex2->var->rstd
                return meanb, sbv

            oute4 = out_e.rearrange("(k p) s -> p k s", p=128)

            def ln_apply_ch(ch, meanb, sbv, lname_s, lname_b, li, triv,
                            final=False, xout_ch=None):
                """Apply one chunk's LN: x = (x2 - mean) * rstd (+affine).
                Non-final+trivial uses 2 coarse DVE ops; final writes the
                fp32 staging tile and DMAs per kt."""
                cs = slice(ch * 512, ch * 512 + 512)
                if not final and triv:
                    # pairs of kt per op: x(kt0,kt1) lands early enough for
                    # the consumer matmuls' first accumulation steps while
                    # keeping the DVE queue short
                    for kt in range(0, NKT, 2):
                        mb = meanb[:, None, cs].to_broadcast([128, 2, 512])
                        sb = sbv[:, None, cs].to_broadcast([128, 2, 512])
                        nc.vector.tensor_sub(xsq_bf[:, kt:kt + 2, cs],
                                             x2_bf[:, kt:kt + 2, cs], mb)
                        nc.vector.tensor_mul(x_bf[:, kt:kt + 2, cs],
                                             xsq_bf[:, kt:kt + 2, cs], sb)
                    return
                for kt in range(NKT):
                    nc.vector.tensor_sub(xsq_bf[:, kt, cs],
                                         x2_bf[:, kt, cs], meanb[:, cs])
                    if final:
                        po = xout_ch[:, kt, :]
                        nc.vector.tensor_mul(po, xsq_bf[:, kt, cs], sbv[:, cs])
                        if not triv:
                            nc.vector.tensor_scalar(
                                out=po, in0=po,
                                scalar1=params[lname_s][:, li, kt:kt + 1],
                                scalar2=params[lname_b][:, li, kt:kt + 1],
                                op0=ALU.mult, op1=ALU.add)
                        nc.sync.dma_start(out=oute4[:, kt, cs], in_=po)
                    else:
                        nc.vector.tensor_mul(x_bf[:, kt, cs],
                                             xsq_bf[:, kt, cs], sbv[:, cs])
                        if not triv:
                            nc.vector.tensor_scalar(
                                out=x_bf[:, kt, cs], in0=x_bf[:, kt, cs],
                                scalar1=params[lname_s][:, li, kt:kt + 1],
                                scalar2=params[lname_b][:, li, kt:kt + 1],
                                op0=ALU.mult, op1=ALU.add)

            def layernorm(lname_s, lname_b, li, triv, filler=None, stats=None):
                """LN over features of x2_bf -> x_bf. Stats from PE
                ones-matmul sums of the same bf16 values the apply uses; pass
                `stats`=(meanb, sbv) when the producing phase already emitted
                per-ch stats."""
                if stats is None:
                    meanb, sbv = new_stats()
                    for ch in range(2):
                        ln_stats_ch(ch, meanb, sbv)
                else:
                    meanb, sbv = stats
                if filler is not None:
                    filler()
                for ch in range(2):
                    ln_apply_ch(ch, meanb, sbv, lname_s, lname_b, li, triv)


            def vproj(li, wv, jts):
                """v = y @ WvT (token-major) -- depends only on y hi/lo + wv,
                so it can fill PE bubbles in other phases. fp8 DoubleRow,
                3-term split: psum holds 32*v, descaled at eviction."""
                wvh, wvl = wv
                vx4 = _vx4_of[li]
                nc.gpsimd.memset(vx4[:, jts.start:jts.stop, :, 64:65], 1.0)
                for jt in jts:
                    pp = ps.tile([128, S], F32, tag="mm")
                    for half in range(2):
                        c0 = half * 256
                        sl = pp[:, c0:c0 + 256]
                        seq = [(y_hi, wvh, 0), (y_hi, wvh, 2),
                               (y_hi, wvl, 0), (y_hi, wvl, 2),
                               (y_lo, wvh, 0), (y_lo, wvh, 2)]
                        for i, (ya, wa, ktp) in enumerate(seq):
                            nc.tensor.matmul(
                                sl, lhsT=ya[:, ktp:ktp + 2, jt * 128:jt * 128 + 128],
                                rhs=wa[:, ktp:ktp + 2, c0:c0 + 256],
                                start=(i == 0),
                                stop=(i == len(seq) - 1) and not has_bv,
                                perf_mode=DR)
                        if has_bv:
                            nc.tensor.matmul(sl, lhsT=ones_row,
                                             rhs=bv_sb[0:1, li, c0:c0 + 256],
                                             start=False, stop=True,
                                             skip_group_check=True)
                    nc.scalar.mul(
                        vx4[:, jt, :, 0:64],
                        pp[:, 0:512].rearrange("p (h c) -> p h c", c=64),
                        1.0 / WV_SCL)
                return vx4

            _vx4_of = {}

            def new_vext(li):
                vext = vp.tile([128, NJT, H * 72], BF16, tag="vext")
                _vx4_of[li] = vext.rearrange("p j (h c) -> p j h c", c=72)
                return _vx4_of[li]

            wv_tiles = {}
            wk_tiles = {}
            wo_tiles = {}

            def load_wv(li, eng):
                th = wqkv.tile([128, NKT, D], F8, tag="wvh")
                eng.dma_start(out=th, in_=wvh_e[li].rearrange("(k p) m -> p k m", p=128))
                tl = wqkv.tile([128, NKT, D], F8, tag="wvl")
                eng.dma_start(out=tl, in_=wvl_e[li].rearrange("(k p) m -> p k m", p=128))
                return th, tl

            def load_wk_wo(li, eng):
                wk = wqkv.tile([128, NKT, D], BF16, tag="wk")
                eng.dma_start(out=wk, in_=wk_e[li].rearrange("(k p) m -> p k m", p=128))
                wo = wqkv.tile([128, NKT, D], BF16, tag="wo")
                eng.dma_start(out=wo, in_=wo_e[li].rearrange("(k p) m -> p k m", p=128))
                return wk, wo

            wff_tiles = {}

            def load_wff(li, eng):
                w1 = wff.tile([128, NKT, FF], BF16, tag="w1")
                eng.dma_start(out=w1, in_=w1_e[li].rearrange("(k p) m -> p k m", p=128))
                w2h = wff2.tile([128, NFT, D], F8, tag="w2h")
                eng.dma_start(out=w2h, in_=w2h_e[li].rearrange("(k p) m -> p k m", p=128))
                w2l = wff2.tile([128, NFT, D], F8, tag="w2l")
                eng.dma_start(out=w2l, in_=w2l_e[li].rearrange("(k p) m -> p k m", p=128))
                return w1, (w2h, w2l)

            nc.sync.dma_start(out=y_hi[:, :, 0:512], in_=yh4[:, :, 0:512])
            nc.sync.dma_start(out=y_lo[:, :, 0:512], in_=yl4[:, :, 0:512])
            wv_tiles[0] = load_wv(0, nc.sync)
            nc.sync.dma_start(out=y_hi[:, :, 512:1024], in_=yh4[:, :, 512:1024])
            nc.sync.dma_start(out=y_lo[:, :, 512:1024], in_=yl4[:, :, 512:1024])
            xT4 = xT_e.rearrange("(k p) s -> p k s", p=128)
            nc.sync.dma_start(out=x_bf[:, :, 0:512], in_=xT4[:, :, 0:512])
            nc.sync.dma_start(out=x_bf[:, :, 512:1024], in_=xT4[:, :, 512:1024])
            wk_tiles[0], wo_tiles[0] = load_wk_wo(0, nc.sync)
            wff_tiles[0] = load_wff(0, nc.sync)

            # ---- constants & params (issued after the critical-path loads)
            params = {}
            mask01 = res.tile([128, 128], BF16, tag="mask")
            nc.gpsimd.dma_start(out=mask01, in_=mask_e[:])
            ones128 = res.tile([128, 128], BF16, tag="ones")
            nc.vector.memset(ones128, 1.0)
            ones_1x64 = res.tile([1, 64], BF16, tag="ones64")
            nc.vector.memset(ones_1x64, 1.0)
            if has_bv:
                ones_row = res.tile([1, 128], BF16, tag="onesr")
                nc.vector.memset(ones_row, 1.0)
                bv_sb = res.tile([1, L, D], BF16, tag="bv")
                nc.gpsimd.dma_start(out=bv_sb, in_=bv_e[:])
            for name, ext, nt in (("bk", bk_e, NKT), ("bo", bo_e, NKT),
                                  ("b1", b1_e, NFT), ("b2", b2_e, NKT),
                                  ("l1s", l1s_e, NKT), ("l1b", l1b_e, NKT),
                                  ("l2s", l2s_e, NKT), ("l2b", l2b_e, NKT)):
                t = res.tile([128, L, nt], F32, tag="prm_" + name)
                nc.gpsimd.dma_start(out=t, in_=ext[:])
                params[name] = t

            new_vext(0)
            vx4 = vproj(0, wv_tiles[0], range(0, NJT))

            for li in range(L):
                wk, wo = wk_tiles[li], wo_tiles[li]
                w1, (w2h, w2l) = wff_tiles[li]
                if li + 1 < L:
                    new_vext(li + 1)
                    # wv early on the same HWDGE queue: its transfer lands
                    # during kq-proj, before the first softmax bounce
                    wv_tiles[li + 1] = load_wv(li + 1, nc.sync)

                # ---- kq projection tile mt feeds heads 2mt, 2mt+1:
                # interleave so exps start after one kq tile and later kq
                # tiles keep the PE busy under the early heads' exps
                def kq_tile(mt):
                    pp = ps.tile([128, S], F32, tag="mm")
                    for ch in range(2):
                        c0 = ch * 512
                        for kt in range(NKT):
                            nc.tensor.matmul(pp[:, c0:c0 + 512],
                                             lhsT=wk[:, kt, mt * 128:mt * 128 + 128],
                                             rhs=x_bf[:, kt, c0:c0 + 512],
                                             start=(kt == 0), stop=(kt == NKT - 1))
                    # DVE eviction: keeps the Act queue clear so head-0 exps
                    # aren't stuck behind three unrelated kq evictions
                    if bk_zero:
                        nc.vector.tensor_copy(kq_bf[:, mt, :], pp)
                    else:
                        nc.vector.tensor_scalar_add(
                            kq_bf[:, mt, :], pp, params["bk"][:, li, mt:mt + 1])

                for mt in range(NKT):
                    kq_tile(mt)

                # ---- attention, head by head
                for h in range(H):
                    po = (h % 2) * 64
                    kqh = kq_bf[po:po + 64, h // 2, :]
                    acc0 = psh.tile([128, 512], F32, tag="mmh")
                    acc1 = psh.tile([128, 512], F32, tag="mmh")
                    # strip groups share one psum tile + one exp: strips
                    # 4..7 are narrow enough to pack pairwise with no padding
                    # (local offsets chosen so no matmul crosses a psum bank)
                    def emit_group(group, locs):
                        sc = ps.tile([128, S], F32, tag="mm")
                        for a in group:
                            for (s0, e0) in _strip_chunks(a):
                                lo = locs[a] + s0 - 128 * a
                                nc.tensor.matmul(sc[:, lo:lo + (e0 - s0)],
                                                 lhsT=kqh[:, 128 * a:128 * a + 128],
                                                 rhs=kqh[:, s0:s0 + (e0 - s0)],
                                                 start=True, stop=True)
                        wtot = max(locs[a] + S - 128 * a for a in group)
                        pt = ptp.tile([128, S], BF16, tag="pt")
                        nc.scalar.activation(out=pt[:, 0:wtot], in_=sc[:, 0:wtot],
                                             func=AF.Exp, scale=float(SCALE))
                        if (len(group) == 2
                                and locs[group[1]] - locs[group[0]] <= 512):
                            stride = locs[group[1]] - locs[group[0]]
                            view = pt[:, 0:stride * len(group)].rearrange(
                                "p (g c) -> p g c", c=stride)
                            nc.vector.tensor_mul(view[:, :, 0:128], view[:, :, 0:128],
                                                 mask01[:, None, :].to_broadcast(
                                                     [128, len(group), 128]))
                        else:
                            for a in group:
                                lo = locs[a]
                                nc.vector.tensor_mul(pt[:, lo:lo + 128],
                                                     pt[:, lo:lo + 128], mask01)
                        return pt

                    def emit_pv(group, locs, pt):
                        # PV needs only the diagonal + absolute-512 splits
                        # (strip-local splits are a scores-psum constraint)
                        for a in group:
                            chunks = [(128 * a, 128 * a + 128)]
                            st = 128 * a + 128
                            for p in (512, S):
                                if st < p <= S:
                                    chunks.append((st, p))
                                    st = p
                            for (s0, e0) in chunks:
                                acc = acc0 if s0 < 512 else acc1
                                o0 = s0 - (0 if s0 < 512 else 512)
                                lo = locs[a] + s0 - 128 * a
                                nc.tensor.matmul(acc[0:65, o0:o0 + (e0 - s0)],
                                                 lhsT=vx4[:, a, h, 0:65],
                                                 rhs=pt[:, lo:lo + (e0 - s0)],
                                                 start=(a == 0),
                                                 stop=(e0 == 128 * a + 128),
                                                 skip_group_check=True)

                    # softmax denominators: bf16 reciprocal straight from the
                    # psum ones-row, col 0 zeroed (empty first query row), then
                    # a DRAM-bounce DMA (HWDGE, low latency) broadcasts it
                    # across 64 partitions and the eviction multiply
                    # normalizes straight out of psum (TT ops may read at most
                    # ONE operand from PSUM, so the broadcast side must be
                    # SBUF). acc0 (cols 0..512) finishes one PV group before
                    # acc1, so its bounce overlaps the last PV group.
                    recip = rcp.tile([1, S], BF16, tag="recip")
                    bnc = dr.tile([1, S], BF16, tag="bounce")
                    rbh = rbp.tile([64, S], BF16, tag="rbh")

                    def norm_half(c0):
                        acc = acc0 if c0 == 0 else acc1
                        cs = slice(c0, c0 + 512)
                        with nc.allow_low_precision(reason="softmax 1/l bf16"):
                            nc.vector.reciprocal(recip[:, cs], acc[64:65, :])
                        if c0 == 0:
                            nc.gpsimd.memset(recip[:, 0:1], 0.0)
                        if h >= H - 2:
                            # last head: local broadcast (PE matmul -> psum
                            # -> SBUF copy) instead of the DRAM bounce; the
                            # ~5us DMA round-trip would be fully exposed at
                            # the phase boundary
                            bps = psh.tile([128, 512], F32, tag="mmh")
                            nc.tensor.matmul(bps[0:64, :], lhsT=ones_1x64,
                                             rhs=recip[0:1, cs],
                                             start=True, stop=True)
                            nc.vector.tensor_copy(rbh[:, cs], bps[0:64, :])
                        else:
                            nc.sync.dma_start(out=bnc[:, cs], in_=recip[:, cs])
                            bap = bnc[:, cs]
                            nc.sync.dma_start(out=rbh[:, cs], in_=bass.AP(
                                tensor=bap.tensor, offset=bap.offset,
                                ap=[[0, 64]] + bap.ap[1:]))

                    def evict_half(c0):
                        acc = acc0 if c0 == 0 else acc1
                        cs = slice(c0, c0 + 512)
                        nc.vector.tensor_mul(outcat[po:po + 64, h // 2, cs],
                                             acc[0:64, :], rbh[:, cs])

                    GROUPS = [([0], {0: 0}), ([1], {1: 0}),
                              ([2, 6], {2: 0, 6: 768}),
                              ([3, 7], {3: 0, 7: 640}),
                              ([4, 5], {4: 0, 5: 512})]
                    pts = [emit_group(*GROUPS[0]), emit_group(*GROUPS[1])]
                    emit_pv(*GROUPS[0], pts[0])
                    pts.append(emit_group(*GROUPS[2]))
                    emit_pv(*GROUPS[1], pts[1])
                    pts.append(emit_group(*GROUPS[3]))
                    emit_pv(*GROUPS[2], pts[2])
                    pts.append(emit_group(*GROUPS[4]))
                    emit_pv(*GROUPS[3], pts[3])
                    norm_half(0)       # acc0 complete: strips 4..7 are acc1-only
                    emit_pv(*GROUPS[4], pts[4])
                    evict_half(0)
                    norm_half(512)
                    evict_half(512)

                # prefetch next layer's weights HERE, on the same in-order
                # HWDGE queue as the softmax bounces: emitted after the head
                # loop, the transfers are forced to queue behind every
                # bounce (cross-queue issue order is NOT program order - the
                # idle Pool sequencer races ahead and its transfers would
                # otherwise block head 0's bounce on the serial DMA engine)
                if li + 1 < L:
                    wk_tiles[li + 1], wo_tiles[li + 1] = load_wk_wo(li + 1, nc.sync)
                    wff_tiles[li + 1] = load_wff(li + 1, nc.sync)
                # last-head bounce filler: two next-layer v-proj token tiles
                if li + 1 < L:
                    vproj(li + 1, wv_tiles[li + 1], range(0, 2))

                # ---- out projection + residual into x2_bf, ch-outer with
                # per-(ch, mt) psum pieces: chunk-0 LN1 stats overlap chunk-1
                # matmuls, hiding most of the LN chain
                ln1_st = new_stats()
                for ch in range(2):
                    c0 = ch * 512
                    cs = slice(c0, c0 + 512)
                    lp = new_lp()
                    for mt in range(NKT):
                        pf = psh.tile([128, 512], F32, tag="mmh", name="po")
                        for kt in range(NKT):
                            nc.tensor.matmul(pf,
                                             lhsT=wo[:, kt, mt * 128:mt * 128 + 128],
                                             rhs=outcat[:, kt, cs],
                                             start=(kt == 0), stop=(kt == NKT - 1))
                        nc.vector.scalar_tensor_tensor(
                            out=x2_bf[:, mt, cs], in0=pf,
                            scalar=params["bo"][:, li, mt:mt + 1],
                            in1=x_bf[:, mt, cs], op0=ALU.add, op1=ALU.add)
                        # square immediately: keeps the LN1 stats chain short
                        nc.scalar.activation(out=xsq_bf[:, mt, cs],
                                             in_=x2_bf[:, mt, cs], func=AF.Square)
                    for kt in range(NKT):
                        ln_stats_mm(lp, kt, ch)
                    ln_stats_fin(lp, ch, *ln1_st)

                fil1 = ((lambda: vproj(li + 1, wv_tiles[li + 1], range(2, 5)))
                        if li + 1 < L else None)
                layernorm("l1s", "l1b", li, ln1_triv, filler=fil1, stats=ln1_st)
                # ffn1-head filler: vproj uses ps-pool psums (disjoint from
                # ffn1's psh pieces) and keeps the PE warm while the LN apply
                # + first h1 evictions drain on DVE/Act
                if li + 1 < L:
                    vproj(li + 1, wv_tiles[li + 1], range(5, 7))

                # ---- ffn1: h1 = relu(W1 @ x + b1), feature-major (bf16
                # matmuls). Evicted directly as an fp8 hi/lo pair for ffn2's
                # DoubleRow matmuls: hi = Act relu (fp8), lo = relu(psum) - hi
                # on DVE. Each (mt, ch) gets its own 1-bank psum piece from
                # psh: psum readers are chained in emission order, so small
                # pieces keep the relu->stt chain at 2 ops and release banks
                # early (the ps-pool [128,S] tiles would serialize 4 readers).
                for mt in range(NFT):
                    for ch in range(2):
                        c0 = ch * 512
                        pf = psh.tile([128, 512], F32, tag="mmh", name="pf")
                        for kt in range(NKT):
                            nc.tensor.matmul(pf,
                                             lhsT=w1[:, kt, mt * 128:mt * 128 + 128],
                                             rhs=x_bf[:, kt, c0:c0 + 512],
                                             start=(kt == 0), stop=(kt == NKT - 1))
                        nc.scalar.activation(out=h1_hi[ch][:, mt, :],
                                             in_=pf, func=AF.Relu,
                                             bias=params["b1"][:, li, mt:mt + 1])
                        nc.vector.scalar_tensor_tensor(
                            out=h1_lo[ch][:, mt, :], in0=pf, scalar=0.0,
                            in1=h1_hi[ch][:, mt, :], op0=ALU.max,
                            op1=ALU.subtract)

                # ---- ffn2 + residual into x2_bf, ch-outer with per-(ch, mt)
                # psum pieces. fp8 DoubleRow 3-term; psum = 2048*ff, descaled
                # in the eviction stt. Chunk-0 LN2 stats (and, on the final
                # layer, chunk-0 apply + output stores) overlap chunk-1.
                final = (li == L - 1)
                ln2_st = new_stats()
                for ch in range(2):
                    cs = slice(ch * 512, ch * 512 + 512)
                    lp = new_lp()
                    for mt in range(NKT):
                        pf = psh.tile([128, 512], F32, tag="mmh", name="pf2")
                        for half in range(2):
                            ci = half * 256
                            sl = pf[:, ci:ci + 256]
                            first = True
                            for ha, wa in ((h1_hi[ch], w2h), (h1_hi[ch], w2l),
                                           (h1_lo[ch], w2h)):
                                for ktp in range(0, NFT, 2):
                                    nc.tensor.matmul(
                                        sl,
                                        lhsT=wa[:, ktp:ktp + 2,
                                                mt * 128:mt * 128 + 128],
                                        rhs=ha[:, ktp:ktp + 2, ci:ci + 256],
                                        start=first,
                                        stop=(ha is h1_lo[ch] and ktp == NFT - 2),
                                        perf_mode=DR)
                                    first = False
                        nc.vector.scalar_tensor_tensor(
                            out=x2_bf[:, mt, cs], in0=pf,
                            scalar=1.0 / W2_SCL,
                            in1=x_bf[:, mt, cs], op0=ALU.mult, op1=ALU.add)
                        if not b2_zero:
                            nc.vector.tensor_scalar_add(
                                x2_bf[:, mt, cs], x2_bf[:, mt, cs],
                                params["b2"][:, li, mt:mt + 1])
                        # square immediately: keeps the LN2 stats chain short
                        nc.scalar.activation(out=xsq_bf[:, mt, cs],
                                             in_=x2_bf[:, mt, cs], func=AF.Square)
                    for kt in range(NKT):
                        ln_stats_mm(lp, kt, ch)
                    ln_stats_fin(lp, ch, *ln2_st)
                    if final:
                        # stream this chunk straight out while the other
                        # chunk's ffn2 matmuls keep the PE busy
                        xout_ch = h1p.tile([128, NKT, 512], F32,
                                           tag=f"h1l{ch}", name=f"xout{ch}")
                        ln_apply_ch(ch, *ln2_st, "l2s", "l2b", li, ln2_triv,
                                    final=True, xout_ch=xout_ch)

                if not final:
                    fil2 = ((lambda: vproj(li + 1, wv_tiles[li + 1],
                                           range(7, NJT)))
                            if li + 1 < L else None)
                    layernorm("l2s", "l2b", li, ln2_triv, filler=fil2,
                              stats=ln2_st)
                    vx4 = _vx4_of[li + 1]

    # Pin every ACT instruction to the one table set that contains all the
    # functions this kernel uses (Exp/Ln/Identity/Relu/Square/Copy), so the
    # whole kernel needs a single ACT_TABLE_LOAD instead of thrashing between
    # the exp- and ln-anchored sets on every layernorm. Indices are preserved
    # (the pass emits act_func_set_id by list position).
    import concourse.bacc as _bacc_mod
    _orig_gat = _bacc_mod.get_activation_tables
    def _pinned_tables(arch):
        tabs = _orig_gat(arch)
        return {name: (funcs if name == "natural_log_exp_and_others" else set())
                for name, funcs in tabs.items()}
    _bacc_mod.get_activation_tables = _pinned_tables
    try:
        nc.compile()
    finally:
        _bacc_mod.get_activation_tables = _orig_gat
    return nc


def _pack_feat(arr, nt):
    """(L, nt*128) fp32 -> [128, L, nt]"""
    Ld = arr.shape[0]
    return np.ascontiguousarray(arr.reshape(Ld, nt, 128).transpose(2, 0, 1)).astype(np.float32)


def _split_f8(a, scale):
    """Split scale*a into fp8e4 hi + lo (hi = rne(x), lo = rne(x - hi))."""
    f8 = ml_dtypes.float8_e4m3
    x = np.clip(np.asarray(a, np.float32) * scale, -240.0, 240.0)
    hi = x.astype(f8)
    lo = np.clip(x - hi.astype(np.float32), -240.0, 240.0).astype(f8)
    return np.ascontiguousarray(hi), np.ascontiguousarray(lo)


def kernel(q_embed_data, qa_embed_data, pe, Wk, bk, Wv, bv, Wo, bo,
           ln1_s, ln1_b, W1, b1, W2, b2, ln2_s, ln2_b, **_unused):
    q = np.asarray(q_embed_data, np.float32)
    qa = np.asarray(qa_embed_data, np.float32)
    pe = np.asarray(pe, np.float32)
    bf = ml_dtypes.bfloat16

    has_bv = bool(np.any(np.asarray(bv)))
    bk_zero = not bool(np.any(np.asarray(bk)))
    b2_zero = not bool(np.any(np.asarray(b2)))
    ln1_triv = bool(np.all(np.asarray(ln1_s) == 1.0) and not np.any(np.asarray(ln1_b)))
    ln2_triv = bool(np.all(np.asarray(ln2_s) == 1.0) and not np.any(np.asarray(ln2_b)))
    key = (has_bv, bk_zero, ln1_triv, ln2_triv, b2_zero)
    if key not in _PROG_CACHE:
        _PROG_CACHE[key] = _build(has_bv, bk_zero, ln1_triv, ln2_triv, b2_zero)
    nc = _PROG_CACHE[key]

    wvT = np.asarray(Wv, np.float32).transpose(0, 2, 1)
    w2T = np.asarray(W2, np.float32).transpose(0, 2, 1)
    wvh, wvl = _split_f8(wvT, WV_SCL)
    w2h, w2l = _split_f8(w2T, W2_SCL)
    shared = {
        "wkT": np.ascontiguousarray(np.asarray(Wk, np.float32).transpose(0, 2, 1)).astype(bf),
        "wvTh": wvh, "wvTl": wvl,
        "woT": np.ascontiguousarray(np.asarray(Wo, np.float32).transpose(0, 2, 1)).astype(bf),
        "w1T": np.ascontiguousarray(np.asarray(W1, np.float32).transpose(0, 2, 1)).astype(bf),
        "w2Th": w2h, "w2Tl": w2l,
        "bkp": _pack_feat(np.asarray(bk, np.float32), NKT),
        "bop": _pack_feat(np.asarray(bo, np.float32), NKT),
        "b1p": _pack_feat(np.asarray(b1, np.float32), NFT),
        "b2p": _pack_feat(np.asarray(b2, np.float32), NKT),
        "l1s": _pack_feat(np.asarray(ln1_s, np.float32), NKT),
        "l1b": _pack_feat(np.asarray(ln1_b, np.float32), NKT),
        "l2s": _pack_feat(np.asarray(ln2_s, np.float32), NKT),
        "l2b": _pack_feat(np.asarray(ln2_b, np.float32), NKT),
        "mask01": (np.arange(128)[:, None] < np.arange(128)[None, :]).astype(bf),
    }
    if has_bv:
        shared["bvp"] = (np.asarray(bv, np.float32) * WV_SCL).reshape(1, L, D).astype(bf)

    in_maps = []
    for c in range(NCORES):
        m = dict(shared)
        m["xT"] = np.ascontiguousarray((q[c] + pe).T).astype(bf)
        yh, yl = _split_f8((qa[c] + pe).T, 1.0)
        m["yTh"] = yh
        m["yTl"] = yl
        in_maps.append(m)

    res = run_bass_kernel_spmd(nc, in_maps, core_ids=list(range(NCORES)))
    out = np.stack([np.ascontiguousarray(res.results[c]["outT"].T)
                    for c in range(NCORES)])
    return out.astype(np.float32)



# revision 51
# speedup vs baseline: 1.0833x; 1.0169x over previous
"""Trainium2 Bass kernel for a 4-layer dense transformer (AKT-style).

Sharding: data-parallel over batch. B=8 batch elements -> 1 per NeuronCore.
Each core runs the full 4-layer stack on its own (S=1024, D=512) slice with
no collectives; weights are replicated.

Per-core layout: feature-major activations [D, S] in bf16 (partition dim =
feature tiles of 128). The residual stream lives entirely in bf16 (x_bf =
post-LN, x2_bf = pre-LN residual); LN statistics come from PE ones-matmul
sums of the same bf16 tensors, so stats and apply are consistent. Matmuls
run in bf16 with fp32 PSUM accumulation. Attention uses the symmetric-scores
trick: S = kq @ kq^T is symmetric, so a [j, i]-layout strip of scores doubles
as the transposed-probabilities operand after a strictly-upper-triangular
causal mask; softmax denominators come from an extra ones-column appended to
V. The per-(head, i) normalizer is computed as a bf16 reciprocal straight
from the PSUM sums row, broadcast across 64 partitions (DRAM-bounce DMA for
heads 0..5, local K=1 PE matmul + SBUF copy for the last two heads where the
DMA round-trip would be exposed), and folded into the PSUM->SBUF eviction
multiply — TT ops may read at most one PSUM operand, so the broadcast side
lives in SBUF. Engine notes: sync=HWDGE carries startup loads, bounces, and
output stores in emission order (weight prefetch queues behind the bounces
on purpose); gpsimd=SWDGE only carries latency-tolerant params. Next-layer
v-projections are spread as PE filler under the softmax-exp (Act-bound) and
LN-stats chains.
"""
import sys

sys.path.insert(0, "/opt/trn_rl_repo")

import math

import ml_dtypes
import numpy as np

import concourse.bass as bass
import concourse.tile as tile
from concourse import bacc, mybir
from concourse.bass_utils import run_bass_kernel_spmd

F32 = mybir.dt.float32
BF16 = mybir.dt.bfloat16
F8 = mybir.dt.float8e4
DR = mybir.MatmulPerfMode.DoubleRow
AF = mybir.ActivationFunctionType
ALU = mybir.AluOpType

# fp8 split scales (powers of 2). Weights are scaled up so both the hi part
# and the hi-lo residual stay clear of the e4m3 subnormal floor; activations
# (x, y, h1) are O(1)..O(32) and need no scaling. psum descale happens at
# eviction.
WV_SCL = 32.0   # v-proj psum = 32 * v
W2_SCL = 64.0   # ffn2 psum = 64 * ff (h1 hi/lo is stored at natural scale)

B, S, D, H, FF, L = 8, 1024, 512, 8, 2048, 4
DK = D // H          # 64
NKT = D // 128       # 4  feature tiles
NJT = S // 128       # 8  token tiles
NFT = FF // 128      # 16 ffn tiles
SCALE = 1.0 / math.sqrt(DK)
EPS = 1e-5
NCORES = 8

_PROG_CACHE = {}


def _strip_chunks(a):
    """Column chunks (absolute i ranges) for scores/PV strip of j-tile a:
    the 128-wide diagonal block first, then pieces that cross neither an
    absolute 512-boundary (PV psum banks) nor a strip-local one (scores
    psum banks, local = absolute - 128*a)."""
    chunks = [(128 * a, 128 * a + 128)]
    start = 128 * a + 128
    pts = sorted({512, 128 * a + 512, S})
    for p in pts:
        if start < p <= S:
            chunks.append((start, p))
            start = p
    return chunks


def _build(has_bv, bk_zero=True, ln1_triv=True, ln2_triv=True, b2_zero=True):
    nc = bacc.Bacc("TRN2", target_bir_lowering=False, debug=False,
                   num_devices=NCORES)

    xT_e = nc.declare_dram_parameter("xT", [D, S], BF16, isOutput=False)
    yh_e = nc.declare_dram_parameter("yTh", [D, S], F8, isOutput=False)
    yl_e = nc.declare_dram_parameter("yTl", [D, S], F8, isOutput=False)
    wk_e = nc.declare_dram_parameter("wkT", [L, D, D], BF16, isOutput=False)
    wvh_e = nc.declare_dram_parameter("wvTh", [L, D, D], F8, isOutput=False)
    wvl_e = nc.declare_dram_parameter("wvTl", [L, D, D], F8, isOutput=False)
    wo_e = nc.declare_dram_parameter("woT", [L, D, D], BF16, isOutput=False)
    w1_e = nc.declare_dram_parameter("w1T", [L, D, FF], BF16, isOutput=False)
    w2h_e = nc.declare_dram_parameter("w2Th", [L, FF, D], F8, isOutput=False)
    w2l_e = nc.declare_dram_parameter("w2Tl", [L, FF, D], F8, isOutput=False)
    # per-feature params packed [128, L, ntiles]
    bk_e = nc.declare_dram_parameter("bkp", [128, L, NKT], F32, isOutput=False)
    bo_e = nc.declare_dram_parameter("bop", [128, L, NKT], F32, isOutput=False)
    b1_e = nc.declare_dram_parameter("b1p", [128, L, NFT], F32, isOutput=False)
    b2_e = nc.declare_dram_parameter("b2p", [128, L, NKT], F32, isOutput=False)
    l1s_e = nc.declare_dram_parameter("l1s", [128, L, NKT], F32, isOutput=False)
    l1b_e = nc.declare_dram_parameter("l1b", [128, L, NKT], F32, isOutput=False)
    l2s_e = nc.declare_dram_parameter("l2s", [128, L, NKT], F32, isOutput=False)
    l2b_e = nc.declare_dram_parameter("l2b", [128, L, NKT], F32, isOutput=False)
    bv_e = nc.declare_dram_parameter("bvp", [1, L, D], BF16, isOutput=False) if has_bv else None
    mask_e = nc.declare_dram_parameter("mask01", [128, 128], BF16, isOutput=False)
    out_e = nc.declare_dram_parameter("outT", [D, S], F32, isOutput=True)

    with tile.TileContext(nc) as tc:
        with (
            tc.tile_pool(name="res", bufs=1) as res,         # resident activations
            tc.tile_pool(name="wqkv", bufs=2) as wqkv,       # per-layer D x D weights
            tc.tile_pool(name="wff", bufs=2) as wff,         # per-layer ffn weights
            tc.tile_pool(name="pt", bufs=5) as ptp,          # exp'd prob strips
            tc.tile_pool(name="vp", bufs=2) as vp,           # v_ext double buffer
            tc.tile_pool(name="rc", bufs=3) as rcp,          # per-head recip rows
            tc.tile_pool(name="bc", bufs=1) as bc,           # LN broadcast tiles
            tc.tile_pool(name="tb", bufs=2) as tbp,
            tc.tile_pool(name="h1p", bufs=1) as h1p,
            tc.tile_pool(name="wff2", bufs=1) as wff2,          # LN apply temps
            tc.tile_pool(name="ps", bufs=2, space="PSUM") as ps,
            tc.tile_pool(name="psh", bufs=4, space="PSUM") as psh,
            tc.tile_pool(name="rb", bufs=3) as rbp,          # recip bcast rows
            tc.tile_pool(name="dr", bufs=4, space="DRAM") as dr,
        ):
            # ---- residents (bf16 residual stream)
            x_bf = res.tile([128, NKT, S], BF16, tag="x_bf")     # post-LN x
            x2_bf = res.tile([128, NKT, S], BF16, tag="x2_bf")   # pre-LN resid
            y_hi = res.tile([128, NKT, S], F8, tag="y_hi")
            y_lo = res.tile([128, NKT, S], F8, tag="y_lo")
            kq_bf = res.tile([128, NKT, S], BF16, tag="kq_bf")
            outcat = res.tile([128, NKT, S], BF16, tag="outcat")
            # h1 fp8 hi/lo, one tile per 512-token chunk: keeps the Act-relu
            # (hi) and DVE-stt (lo) evictions of different chunks free of
            # false subtile WARs (tracking is first-free-dim granular)
            h1_hi = [h1p.tile([128, NFT, 512], F8, tag=f"h1h{c}", name=f"h1h{c}")
                     for c in range(2)]
            h1_lo = [h1p.tile([128, NFT, 512], F8, tag=f"h1l{c}", name=f"h1l{c}")
                     for c in range(2)]
            xsq_bf = res.tile([128, NKT, S], BF16, tag="xsq")

            # ---- initial loads: x = q+pe, y = qa+pe precomputed on host.
            # sync = HWDGE (fast, low latency): carries the startup-critical
            # loads in need order (y+wv feed vproj first), then only the
            # latency-sensitive softmax bounces + output stores.
            # gpsimd = SWDGE (Q7 descgen ~1us, latency-tolerant): params,
            # mask, and all next-layer weight prefetches.
            yh4 = yh_e.rearrange("(k p) s -> p k s", p=128)
            yl4 = yl_e.rearrange("(k p) s -> p k s", p=128)

            def ln_stats_mm(lp, kt, ch):
                """One accumulation step of the per-chunk stats sums; emit
                kt lagged behind the producer's evict of feature tile kt so
                it never head-of-line-blocks the producer's own matmuls."""
                lp0, lp1 = lp
                cs = slice(ch * 512, ch * 512 + 512)
                nc.tensor.matmul(lp0, lhsT=ones128, rhs=x2_bf[:, kt, cs],
                                 start=(kt == 0), stop=(kt == NKT - 1))
                nc.tensor.matmul(lp1, lhsT=ones128, rhs=xsq_bf[:, kt, cs],
                                 start=(kt == 0), stop=(kt == NKT - 1))

            def new_lp():
                return (psh.tile([128, 512], F32, tag="mmh", name="lp0"),
                        psh.tile([128, 512], F32, tag="mmh", name="lp1"))

            def ln_stats_fin(lp, ch, meanb, sbv):
                """Finish one chunk's stats: mean/E[x2] eviction split across
                Act (mean, mean^2) and DVE (E[x2], var) so the serial chain is
                ~1us shorter than an all-DVE version; rstd via Ln/Exp."""
                lp0, lp1 = lp
                cs = slice(ch * 512, ch * 512 + 512)
                nc.scalar.mul(meanb[:, cs], lp0, 1.0 / D)
                nc.vector.tensor_scalar_mul(sbv[:, cs], lp1, 1.0 / D)
                m2 = tbp.tile([128, 512], BF16, tag="lntmp")
                nc.scalar.square(m2, meanb[:, cs])
                nc.vector.scalar_tensor_tensor(out=sbv[:, cs], in0=sbv[:, cs],
                                               scalar=float(EPS), in1=m2,
                                               op0=ALU.add, op1=ALU.subtract)
                nc.scalar.activation(out=sbv[:, cs], in_=sbv[:, cs], func=AF.Ln)
                nc.scalar.activation(out=sbv[:, cs], in_=sbv[:, cs],
                                     func=AF.Exp, scale=-0.5)

            def ln_stats_ch(ch, meanb, sbv):
                lp = new_lp()
                for kt in range(NKT):
                    ln_stats_mm(lp, kt, ch)
                ln_stats_fin(lp, ch, meanb, sbv)

            def new_stats():
                meanb = bc.tile([128, S], BF16, tag="meanb")
                sbv = bc.tile([128, S], BF16, tag="statb")  # ex2->var->rstd
                return meanb, sbv

            oute4 = out_e.rearrange("(k p) s -> p k s", p=128)

            def ln_apply_ch(ch, meanb, sbv, lname_s, lname_b, li, triv,
                            final=False, xout_ch=None):
                """Apply one chunk's LN: x = (x2 - mean) * rstd (+affine).
                Non-final+trivial uses 2 coarse DVE ops; final writes the
                fp32 staging tile and DMAs per kt."""
                cs = slice(ch * 512, ch * 512 + 512)
                if not final and triv:
                    # pairs of kt per op: x(kt0,kt1) lands early enough for
                    # the consumer matmuls' first accumulation steps while
                    # keeping the DVE queue short
                    for kt in range(0, NKT, 2):
                        mb = meanb[:, None, cs].to_broadcast([128, 2, 512])
                        sb = sbv[:, None, cs].to_broadcast([128, 2, 512])
                        nc.vector.tensor_sub(xsq_bf[:, kt:kt + 2, cs],
                                             x2_bf[:, kt:kt + 2, cs], mb)
                        nc.vector.tensor_mul(x_bf[:, kt:kt + 2, cs],
                                             xsq_bf[:, kt:kt + 2, cs], sb)
                    return
                for kt in range(NKT):
                    nc.vector.tensor_sub(xsq_bf[:, kt, cs],
                                         x2_bf[:, kt, cs], meanb[:, cs])
                    if final:
                        po = xout_ch[:, kt, :]
                        nc.vector.tensor_mul(po, xsq_bf[:, kt, cs], sbv[:, cs])
                        if not triv:
                            nc.vector.tensor_scalar(
                                out=po, in0=po,
                                scalar1=params[lname_s][:, li, kt:kt + 1],
                                scalar2=params[lname_b][:, li, kt:kt + 1],
                                op0=ALU.mult, op1=ALU.add)
                        nc.sync.dma_start(out=oute4[:, kt, cs], in_=po)
                    else:
                        nc.vector.tensor_mul(x_bf[:, kt, cs],
                                             xsq_bf[:, kt, cs], sbv[:, cs])
                        if not triv:
                            nc.vector.tensor_scalar(
                                out=x_bf[:, kt, cs], in0=x_bf[:, kt, cs],
                                scalar1=params[lname_s][:, li, kt:kt + 1],
                                scalar2=params[lname_b][:, li, kt:kt + 1],
                                op0=ALU.mult, op1=ALU.add)

            def layernorm(lname_s, lname_b, li, triv, filler=None, stats=None):
                """LN over features of x2_bf -> x_bf. Stats from PE
                ones-matmul sums of the same bf16 values the apply uses; pass
                `stats`=(meanb, sbv) when the producing phase already emitted
                per-ch stats."""
                if stats is None:
                    meanb, sbv = new_stats()
                    for ch in range(2):
                        ln_stats_ch(ch, meanb, sbv)
                else:
                    meanb, sbv = stats
                if filler is not None:
                    filler()
                for ch in range(2):
                    ln_apply_ch(ch, meanb, sbv, lname_s, lname_b, li, triv)


            def vproj(li, wv, jts):
                """v = y @ WvT (token-major) -- depends only on y hi/lo + wv,
                so it can fill PE bubbles in other phases. fp8 DoubleRow,
                3-term split: psum holds 32*v, descaled at eviction."""
                wvh, wvl = wv
                vx4 = _vx4_of[li]
                nc.gpsimd.memset(vx4[:, jts.start:jts.stop, :, 64:65], 1.0)
                for jt in jts:
                    pp = ps.tile([128, S], F32, tag="mm")
                    for half in range(2):
                        c0 = half * 256
                        sl = pp[:, c0:c0 + 256]
                        seq = [(y_hi, wvh, 0), (y_hi, wvh, 2),
                               (y_hi, wvl, 0), (y_hi, wvl, 2),
                               (y_lo, wvh, 0), (y_lo, wvh, 2)]
                        for i, (ya, wa, ktp) in enumerate(seq):
                            nc.tensor.matmul(
                                sl, lhsT=ya[:, ktp:ktp + 2, jt * 128:jt * 128 + 128],
                                rhs=wa[:, ktp:ktp + 2, c0:c0 + 256],
                                start=(i == 0),
                                stop=(i == len(seq) - 1) and not has_bv,
                                perf_mode=DR)
                        if has_bv:
                            nc.tensor.matmul(sl, lhsT=ones_row,
                                             rhs=bv_sb[0:1, li, c0:c0 + 256],
                                             start=False, stop=True,
                                             skip_group_check=True)
                    nc.scalar.mul(
                        vx4[:, jt, :, 0:64],
                        pp[:, 0:512].rearrange("p (h c) -> p h c", c=64),
                        1.0 / WV_SCL)
                return vx4

            _vx4_of = {}

            def new_vext(li):
                vext = vp.tile([128, NJT, H * 72], BF16, tag="vext")
                _vx4_of[li] = vext.rearrange("p j (h c) -> p j h c", c=72)
                return _vx4_of[li]

            wv_tiles = {}
            wk_tiles = {}
            wo_tiles = {}

            def load_wv(li, eng):
                th = wqkv.tile([128, NKT, D], F8, tag="wvh")
                eng.dma_start(out=th, in_=wvh_e[li].rearrange("(k p) m -> p k m", p=128))
                tl = wqkv.tile([128, NKT, D], F8, tag="wvl")
                eng.dma_start(out=tl, in_=wvl_e[li].rearrange("(k p) m -> p k m", p=128))
                return th, tl

            def load_wk_wo(li, eng):
                wk = wqkv.tile([128, NKT, D], BF16, tag="wk")
                eng.dma_start(out=wk, in_=wk_e[li].rearrange("(k p) m -> p k m", p=128))
                wo = wqkv.tile([128, NKT, D], BF16, tag="wo")
                eng.dma_start(out=wo, in_=wo_e[li].rearrange("(k p) m -> p k m", p=128))
                return wk, wo

            wff_tiles = {}

            def load_wff(li, eng):
                w1 = wff.tile([128, NKT, FF], BF16, tag="w1")
                eng.dma_start(out=w1, in_=w1_e[li].rearrange("(k p) m -> p k m", p=128))
                w2h = wff2.tile([128, NFT, D], F8, tag="w2h")
                eng.dma_start(out=w2h, in_=w2h_e[li].rearrange("(k p) m -> p k m", p=128))
                w2l = wff2.tile([128, NFT, D], F8, tag="w2l")
                eng.dma_start(out=w2l, in_=w2l_e[li].rearrange("(k p) m -> p k m", p=128))
                return w1, (w2h, w2l)

            nc.sync.dma_start(out=y_hi[:, :, 0:512], in_=yh4[:, :, 0:512])
            nc.sync.dma_start(out=y_lo[:, :, 0:512], in_=yl4[:, :, 0:512])
            wv_tiles[0] = load_wv(0, nc.sync)
            nc.sync.dma_start(out=y_hi[:, :, 512:1024], in_=yh4[:, :, 512:1024])
            nc.sync.dma_start(out=y_lo[:, :, 512:1024], in_=yl4[:, :, 512:1024])
            xT4 = xT_e.rearrange("(k p) s -> p k s", p=128)
            nc.sync.dma_start(out=x_bf[:, :, 0:512], in_=xT4[:, :, 0:512])
            nc.sync.dma_start(out=x_bf[:, :, 512:1024], in_=xT4[:, :, 512:1024])
            wk_tiles[0], wo_tiles[0] = load_wk_wo(0, nc.sync)
            wff_tiles[0] = load_wff(0, nc.sync)

            # ---- constants & params (issued after the critical-path loads)
            params = {}
            mask01 = res.tile([128, 128], BF16, tag="mask")
            nc.gpsimd.dma_start(out=mask01, in_=mask_e[:])
            ones128 = res.tile([128, 128], BF16, tag="ones")
            nc.vector.memset(ones128, 1.0)
            ones_1x64 = res.tile([1, 64], BF16, tag="ones64")
            nc.vector.memset(ones_1x64, 1.0)
            if has_bv:
                ones_row = res.tile([1, 128], BF16, tag="onesr")
                nc.vector.memset(ones_row, 1.0)
                bv_sb = res.tile([1, L, D], BF16, tag="bv")
                nc.gpsimd.dma_start(out=bv_sb, in_=bv_e[:])
            for name, ext, nt in (("bk", bk_e, NKT), ("bo", bo_e, NKT),
                                  ("b1", b1_e, NFT), ("b2", b2_e, NKT),
                                  ("l1s", l1s_e, NKT), ("l1b", l1b_e, NKT),
                                  ("l2s", l2s_e, NKT), ("l2b", l2b_e, NKT)):
                t = res.tile([128, L, nt], F32, tag="prm_" + name)
                nc.gpsimd.dma_start(out=t, in_=ext[:])
                params[name] = t

            new_vext(0)
            vx4 = vproj(0, wv_tiles[0], range(0, NJT))

            for li in range(L):
                wk, wo = wk_tiles[li], wo_tiles[li]
                w1, (w2h, w2l) = wff_tiles[li]
                if li + 1 < L:
                    new_vext(li + 1)
                    # wv early on the same HWDGE queue: its transfer lands
                    # during kq-proj, before the first softmax bounce
                    wv_tiles[li + 1] = load_wv(li + 1, nc.sync)

                # ---- kq projection tile mt feeds heads 2mt, 2mt+1:
                # interleave so exps start after one kq tile and later kq
                # tiles keep the PE busy under the early heads' exps
                def kq_tile(mt):
                    pp = ps.tile([128, S], F32, tag="mm")
                    for ch in range(2):
                        c0 = ch * 512
                        for kt in range(NKT):
                            nc.tensor.matmul(pp[:, c0:c0 + 512],
                                             lhsT=wk[:, kt, mt * 128:mt * 128 + 128],
                                             rhs=x_bf[:, kt, c0:c0 + 512],
                                             start=(kt == 0), stop=(kt == NKT - 1))
                    # DVE eviction: keeps the Act queue clear so head-0 exps
                    # aren't stuck behind three unrelated kq evictions
                    if bk_zero:
                        nc.vector.tensor_copy(kq_bf[:, mt, :], pp)
                    else:
                        nc.vector.tensor_scalar_add(
                            kq_bf[:, mt, :], pp, params["bk"][:, li, mt:mt + 1])

                for mt in range(NKT):
                    kq_tile(mt)

                # ---- attention, head by head
                for h in range(H):
                    po = (h % 2) * 64
                    kqh = kq_bf[po:po + 64, h // 2, :]
                    acc0 = psh.tile([128, 512], F32, tag="mmh")
                    acc1 = psh.tile([128, 512], F32, tag="mmh")
                    # strip groups share one psum tile + one exp: strips
                    # 4..7 are narrow enough to pack pairwise with no padding
                    # (local offsets chosen so no matmul crosses a psum bank)
                    def emit_group(group, locs):
                        sc = ps.tile([128, S], F32, tag="mm")
                        for a in group:
                            for (s0, e0) in _strip_chunks(a):
                                lo = locs[a] + s0 - 128 * a
                                nc.tensor.matmul(sc[:, lo:lo + (e0 - s0)],
                                                 lhsT=kqh[:, 128 * a:128 * a + 128],
                                                 rhs=kqh[:, s0:s0 + (e0 - s0)],
                                                 start=True, stop=True)
                        wtot = max(locs[a] + S - 128 * a for a in group)
                        pt = ptp.tile([128, S], BF16, tag="pt")
                        nc.scalar.activation(out=pt[:, 0:wtot], in_=sc[:, 0:wtot],
                                             func=AF.Exp, scale=float(SCALE))
                        if (len(group) == 2
                                and locs[group[1]] - locs[group[0]] <= 512):
                            stride = locs[group[1]] - locs[group[0]]
                            view = pt[:, 0:stride * len(group)].rearrange(
                                "p (g c) -> p g c", c=stride)
                            nc.vector.tensor_mul(view[:, :, 0:128], view[:, :, 0:128],
                                                 mask01[:, None, :].to_broadcast(
                                                     [128, len(group), 128]))
                        else:
                            for a in group:
                                lo = locs[a]
                                nc.vector.tensor_mul(pt[:, lo:lo + 128],
                                                     pt[:, lo:lo + 128], mask01)
                        return pt

                    def emit_pv(group, locs, pt):
                        # PV needs only the diagonal + absolute-512 splits
                        # (strip-local splits are a scores-psum constraint)
                        for a in group:
                            chunks = [(128 * a, 128 * a + 128)]
                            st = 128 * a + 128
                            for p in (512, S):
                                if st < p <= S:
                                    chunks.append((st, p))
                                    st = p
                            for (s0, e0) in chunks:
                                acc = acc0 if s0 < 512 else acc1
                                o0 = s0 - (0 if s0 < 512 else 512)
                                lo = locs[a] + s0 - 128 * a
                                nc.tensor.matmul(acc[0:65, o0:o0 + (e0 - s0)],
                                                 lhsT=vx4[:, a, h, 0:65],
                                                 rhs=pt[:, lo:lo + (e0 - s0)],
                                                 start=(a == 0),
                                                 stop=(e0 == 128 * a + 128),
                                                 skip_group_check=True)

                    # softmax denominators: bf16 reciprocal straight from the
                    # psum ones-row, col 0 zeroed (empty first query row), then
                    # a DRAM-bounce DMA (HWDGE, low latency) broadcasts it
                    # across 64 partitions and the eviction multiply
                    # normalizes straight out of psum (TT ops may read at most
                    # ONE operand from PSUM, so the broadcast side must be
                    # SBUF). acc0 (cols 0..512) finishes one PV group before
                    # acc1, so its bounce overlaps the last PV group.
                    recip = rcp.tile([1, S], BF16, tag="recip")
                    bnc = dr.tile([1, S], BF16, tag="bounce")
                    rbh = rbp.tile([64, S], BF16, tag="rbh")

                    def norm_half(c0):
                        acc = acc0 if c0 == 0 else acc1
                        cs = slice(c0, c0 + 512)
                        with nc.allow_low_precision(reason="softmax 1/l bf16"):
                            nc.vector.reciprocal(recip[:, cs], acc[64:65, :])
                        if c0 == 0:
                            nc.gpsimd.memset(recip[:, 0:1], 0.0)
                        if h >= H - 2:
                            # last head: local broadcast (PE matmul -> psum
                            # -> SBUF copy) instead of the DRAM bounce; the
                            # ~5us DMA round-trip would be fully exposed at
                            # the phase boundary
                            bps = psh.tile([128, 512], F32, tag="mmh")
                            nc.tensor.matmul(bps[0:64, :], lhsT=ones_1x64,
                                             rhs=recip[0:1, cs],
                                             start=True, stop=True)
                            nc.vector.tensor_copy(rbh[:, cs], bps[0:64, :])
                        else:
                            nc.sync.dma_start(out=bnc[:, cs], in_=recip[:, cs])
                            bap = bnc[:, cs]
                            nc.sync.dma_start(out=rbh[:, cs], in_=bass.AP(
                                tensor=bap.tensor, offset=bap.offset,
                                ap=[[0, 64]] + bap.ap[1:]))

                    def evict_half(c0):
                        acc = acc0 if c0 == 0 else acc1
                        cs = slice(c0, c0 + 512)
                        nc.vector.tensor_mul(outcat[po:po + 64, h // 2, cs],
                                             acc[0:64, :], rbh[:, cs])

                    GROUPS = [([0], {0: 0}), ([1], {1: 0}),
                              ([2, 6], {2: 0, 6: 768}),
                              ([3, 7], {3: 0, 7: 640}),
                              ([4, 5], {4: 0, 5: 512})]
                    pts = [emit_group(*GROUPS[0]), emit_group(*GROUPS[1])]
                    emit_pv(*GROUPS[0], pts[0])
                    pts.append(emit_group(*GROUPS[2]))
                    emit_pv(*GROUPS[1], pts[1])
                    pts.append(emit_group(*GROUPS[3]))
                    emit_pv(*GROUPS[2], pts[2])
                    pts.append(emit_group(*GROUPS[4]))
                    emit_pv(*GROUPS[3], pts[3])
                    norm_half(0)       # acc0 complete: strips 4..7 are acc1-only
                    emit_pv(*GROUPS[4], pts[4])
                    evict_half(0)
                    norm_half(512)
                    evict_half(512)

                # prefetch next layer's weights HERE, on the same in-order
                # HWDGE queue as the softmax bounces: emitted after the head
                # loop, the transfers are forced to queue behind every
                # bounce (cross-queue issue order is NOT program order - the
                # idle Pool sequencer races ahead and its transfers would
                # otherwise block head 0's bounce on the serial DMA engine)
                if li + 1 < L:
                    wk_tiles[li + 1], wo_tiles[li + 1] = load_wk_wo(li + 1, nc.sync)
                    wff_tiles[li + 1] = load_wff(li + 1, nc.sync)
                # last-head bounce filler: two next-layer v-proj token tiles
                if li + 1 < L:
                    vproj(li + 1, wv_tiles[li + 1], range(0, 2))

                # ---- out projection + residual into x2_bf, ch-outer with
                # per-(ch, mt) psum pieces: chunk-0 LN1 stats overlap chunk-1
                # matmuls, hiding most of the LN chain
                ln1_st = new_stats()
                for ch in range(2):
                    c0 = ch * 512
                    cs = slice(c0, c0 + 512)
                    lp = new_lp()
                    for mt in range(NKT):
                        pf = psh.tile([128, 512], F32, tag="mmh", name="po")
                        for kt in range(NKT):
                            nc.tensor.matmul(pf,
                                             lhsT=wo[:, kt, mt * 128:mt * 128 + 128],
                                             rhs=outcat[:, kt, cs],
                                             start=(kt == 0), stop=(kt == NKT - 1))
                        nc.vector.scalar_tensor_tensor(
                            out=x2_bf[:, mt, cs], in0=pf,
                            scalar=params["bo"][:, li, mt:mt + 1],
                            in1=x_bf[:, mt, cs], op0=ALU.add, op1=ALU.add)
                        # square immediately: keeps the LN1 stats chain short
                        nc.scalar.activation(out=xsq_bf[:, mt, cs],
                                             in_=x2_bf[:, mt, cs], func=AF.Square)
                    for kt in range(NKT):
                        ln_stats_mm(lp, kt, ch)
                    ln_stats_fin(lp, ch, *ln1_st)

                fil1 = ((lambda: vproj(li + 1, wv_tiles[li + 1], range(2, 5)))
                        if li + 1 < L else None)
                layernorm("l1s", "l1b", li, ln1_triv, filler=fil1, stats=ln1_st)
                # ffn1-head filler: vproj uses ps-pool psums (disjoint from
                # ffn1's psh pieces) and keeps the PE warm while the LN apply
                # + first h1 evictions drain on DVE/Act
                if li + 1 < L:
                    vproj(li + 1, wv_tiles[li + 1], range(5, 7))

                # ---- ffn1: h1 = relu(W1 @ x + b1), feature-major (bf16
                # matmuls). Evicted directly as an fp8 hi/lo pair for ffn2's
                # DoubleRow matmuls: hi = Act relu (fp8), lo = relu(psum) - hi
                # on DVE. Each (mt, ch) gets its own 1-bank psum piece from
                # psh: psum readers are chained in emission order, so small
                # pieces keep the relu->stt chain at 2 ops and release banks
                # early (the ps-pool [128,S] tiles would serialize 4 readers).
                for ch in range(2):
                    for mt in range(NFT):
                        c0 = ch * 512
                        pf = psh.tile([128, 512], F32, tag="mmh", name="pf")
                        for kt in range(NKT):
                            nc.tensor.matmul(pf,
                                             lhsT=w1[:, kt, mt * 128:mt * 128 + 128],
                                             rhs=x_bf[:, kt, c0:c0 + 512],
                                             start=(kt == 0), stop=(kt == NKT - 1))
                        nc.scalar.activation(out=h1_hi[ch][:, mt, :],
                                             in_=pf, func=AF.Relu,
                                             bias=params["b1"][:, li, mt:mt + 1])
                        nc.vector.scalar_tensor_tensor(
                            out=h1_lo[ch][:, mt, :], in0=pf, scalar=0.0,
                            in1=h1_hi[ch][:, mt, :], op0=ALU.max,
                            op1=ALU.subtract)

                # ---- ffn2 + residual into x2_bf, ch-outer with per-(ch, mt)
                # psum pieces. fp8 DoubleRow 3-term; psum = 2048*ff, descaled
                # in the eviction stt. Chunk-0 LN2 stats (and, on the final
                # layer, chunk-0 apply + output stores) overlap chunk-1.
                final = (li == L - 1)
                ln2_st = new_stats()
                for ch in range(2):
                    cs = slice(ch * 512, ch * 512 + 512)
                    lp = new_lp()
                    for mt in range(NKT):
                        pf = psh.tile([128, 512], F32, tag="mmh", name="pf2")
                        for half in range(2):
                            ci = half * 256
                            sl = pf[:, ci:ci + 256]
                            first = True
                            for ha, wa in ((h1_hi[ch], w2h), (h1_hi[ch], w2l),
                                           (h1_lo[ch], w2h)):
                                for ktp in range(0, NFT, 2):
                                    nc.tensor.matmul(
                                        sl,
                                        lhsT=wa[:, ktp:ktp + 2,
                                                mt * 128:mt * 128 + 128],
                                        rhs=ha[:, ktp:ktp + 2, ci:ci + 256],
                                        start=first,
                                        stop=(ha is h1_lo[ch] and ktp == NFT - 2),
                                        perf_mode=DR)
                                    first = False
                        nc.vector.scalar_tensor_tensor(
                            out=x2_bf[:, mt, cs], in0=pf,
                            scalar=1.0 / W2_SCL,
                            in1=x_bf[:, mt, cs], op0=ALU.mult, op1=ALU.add)
                        if not b2_zero:
                            nc.vector.tensor_scalar_add(
                                x2_bf[:, mt, cs], x2_bf[:, mt, cs],
                                params["b2"][:, li, mt:mt + 1])
                        # square immediately: keeps the LN2 stats chain short
                        nc.scalar.activation(out=xsq_bf[:, mt, cs],
                                             in_=x2_bf[:, mt, cs], func=AF.Square)
                    for kt in range(NKT):
                        ln_stats_mm(lp, kt, ch)
                    ln_stats_fin(lp, ch, *ln2_st)
                    if final:
                        # stream this chunk straight out while the other
                        # chunk's ffn2 matmuls keep the PE busy
                        xout_ch = h1p.tile([128, NKT, 512], F32,
                                           tag=f"h1l{ch}", name=f"xout{ch}")
                        ln_apply_ch(ch, *ln2_st, "l2s", "l2b", li, ln2_triv,
                                    final=True, xout_ch=xout_ch)

                if not final:
                    fil2 = ((lambda: vproj(li + 1, wv_tiles[li + 1],
                                           range(7, NJT)))
                            if li + 1 < L else None)
                    layernorm("l2s", "l2b", li, ln2_triv, filler=fil2,
                              stats=ln2_st)
                    vx4 = _vx4_of[li + 1]

    # Pin every ACT instruction to the one table set that contains all the
    # functions this kernel uses (Exp/Ln/Identity/Relu/Square/Copy), so the
    # whole kernel needs a single ACT_TABLE_LOAD instead of thrashing between
    # the exp- and ln-anchored sets on every layernorm. Indices are preserved
    # (the pass emits act_func_set_id by list position).
    import concourse.bacc as _bacc_mod
    _orig_gat = _bacc_mod.get_activation_tables
    def _pinned_tables(arch):
        tabs = _orig_gat(arch)
        return {name: (funcs if name == "natural_log_exp_and_others" else set())
                for name, funcs in tabs.items()}
    _bacc_mod.get_activation_tables = _pinned_tables
    try:
        nc.compile()
    finally:
        _bacc_mod.get_activation_tables = _orig_gat
    return nc


def _pack_feat(arr, nt):
    """(L, nt*128) fp32 -> [128, L, nt]"""
    Ld = arr.shape[0]
    return np.ascontiguousarray(arr.reshape(Ld, nt, 128).transpose(2, 0, 1)).astype(np.float32)


def _split_f8(a, scale):
    """Split scale*a into fp8e4 hi + lo (hi = rne(x), lo = rne(x - hi))."""
    f8 = ml_dtypes.float8_e4m3
    x = np.clip(np.asarray(a, np.float32) * scale, -240.0, 240.0)
    hi = x.astype(f8)
    lo = np.clip(x - hi.astype(np.float32), -240.0, 240.0).astype(f8)
    return np.ascontiguousarray(hi), np.ascontiguousarray(lo)


def kernel(q_embed_data, qa_embed_data, pe, Wk, bk, Wv, bv, Wo, bo,
           ln1_s, ln1_b, W1, b1, W2, b2, ln2_s, ln2_b, **_unused):
    q = np.asarray(q_embed_data, np.float32)
    qa = np.asarray(qa_embed_data, np.float32)
    pe = np.asarray(pe, np.float32)
    bf = ml_dtypes.bfloat16

    has_bv = bool(np.any(np.asarray(bv)))
    bk_zero = not bool(np.any(np.asarray(bk)))
    b2_zero = not bool(np.any(np.asarray(b2)))
    ln1_triv = bool(np.all(np.asarray(ln1_s) == 1.0) and not np.any(np.asarray(ln1_b)))
    ln2_triv = bool(np.all(np.asarray(ln2_s) == 1.0) and not np.any(np.asarray(ln2_b)))
    key = (has_bv, bk_zero, ln1_triv, ln2_triv, b2_zero)
    if key not in _PROG_CACHE:
        _PROG_CACHE[key] = _build(has_bv, bk_zero, ln1_triv, ln2_triv, b2_zero)
    nc = _PROG_CACHE[key]

    wvT = np.asarray(Wv, np.float32).transpose(0, 2, 1)
    w2T = np.asarray(W2, np.float32).transpose(0, 2, 1)
    wvh, wvl = _split_f8(wvT, WV_SCL)
    w2h, w2l = _split_f8(w2T, W2_SCL)
    shared = {
        "wkT": np.ascontiguousarray(np.asarray(Wk, np.float32).transpose(0, 2, 1)).astype(bf),
        "wvTh": wvh, "wvTl": wvl,
        "woT": np.ascontiguousarray(np.asarray(Wo, np.float32).transpose(0, 2, 1)).astype(bf),
        "w1T": np.ascontiguousarray(np.asarray(W1, np.float32).transpose(0, 2, 1)).astype(bf),
        "w2Th": w2h, "w2Tl": w2l,
        "bkp": _pack_feat(np.asarray(bk, np.float32), NKT),
        "bop": _pack_feat(np.asarray(bo, np.float32), NKT),
        "b1p": _pack_feat(np.asarray(b1, np.float32), NFT),
        "b2p": _pack_feat(np.asarray(b2, np.float32), NKT),
        "l1s": _pack_feat(np.asarray(ln1_s, np.float32), NKT),
        "l1b": _pack_feat(np.asarray(ln1_b, np.float32), NKT),
        "l2s": _pack_feat(np.asarray(ln2_s, np.float32), NKT),
        "l2b": _pack_feat(np.asarray(ln2_b, np.float32), NKT),
        "mask01": (np.arange(128)[:, None] < np.arange(128)[None, :]).astype(bf),
    }
    if has_bv:
        shared["bvp"] = (np.asarray(bv, np.float32) * WV_SCL).reshape(1, L, D).astype(bf)

    in_maps = []
    for c in range(NCORES):
        m = dict(shared)
        m["xT"] = np.ascontiguousarray((q[c] + pe).T).astype(bf)
        yh, yl = _split_f8((qa[c] + pe).T, 1.0)
        m["yTh"] = yh
        m["yTl"] = yl
        in_maps.append(m)

    res = run_bass_kernel_spmd(nc, in_maps, core_ids=list(range(NCORES)))
    out = np.stack([np.ascontiguousarray(res.results[c]["outT"].T)
                    for c in range(NCORES)])
    return out.astype(np.float32)



# revision 52
# speedup vs baseline: 1.0993x; 1.0148x over previous
"""Trainium2 Bass kernel for a 4-layer dense transformer (AKT-style).

Sharding: data-parallel over batch. B=8 batch elements -> 1 per NeuronCore.
Each core runs the full 4-layer stack on its own (S=1024, D=512) slice with
no collectives; weights are replicated.

Per-core layout: feature-major activations [D, S] in bf16 (partition dim =
feature tiles of 128). The residual stream lives entirely in bf16 (x_bf =
post-LN, x2_bf = pre-LN residual); LN statistics come from PE ones-matmul
sums of the same bf16 tensors, so stats and apply are consistent. Matmuls
run in bf16 with fp32 PSUM accumulation. Attention uses the symmetric-scores
trick: S = kq @ kq^T is symmetric, so a [j, i]-layout strip of scores doubles
as the transposed-probabilities operand after a strictly-upper-triangular
causal mask; softmax denominators come from an extra ones-column appended to
V. The per-(head, i) normalizer is computed as a bf16 reciprocal straight
from the PSUM sums row, broadcast across 64 partitions (DRAM-bounce DMA for
heads 0..5, local K=1 PE matmul + SBUF copy for the last two heads where the
DMA round-trip would be exposed), and folded into the PSUM->SBUF eviction
multiply — TT ops may read at most one PSUM operand, so the broadcast side
lives in SBUF. Engine notes: sync=HWDGE carries startup loads, bounces, and
output stores in emission order (weight prefetch queues behind the bounces
on purpose); gpsimd=SWDGE only carries latency-tolerant params. Next-layer
v-projections are spread as PE filler under the softmax-exp (Act-bound) and
LN-stats chains.
"""
import sys

sys.path.insert(0, "/opt/trn_rl_repo")

import math

import ml_dtypes
import numpy as np

import concourse.bass as bass
import concourse.tile as tile
from concourse import bacc, mybir
from concourse.bass_utils import run_bass_kernel_spmd

F32 = mybir.dt.float32
BF16 = mybir.dt.bfloat16
F8 = mybir.dt.float8e4
DR = mybir.MatmulPerfMode.DoubleRow
AF = mybir.ActivationFunctionType
ALU = mybir.AluOpType

# fp8 split scales (powers of 2). Weights are scaled up so both the hi part
# and the hi-lo residual stay clear of the e4m3 subnormal floor; activations
# (x, y, h1) are O(1)..O(32) and need no scaling. psum descale happens at
# eviction.
WV_SCL = 32.0   # v-proj psum = 32 * v
W2_SCL = 64.0   # ffn2 psum = 64 * ff (h1 hi/lo is stored at natural scale)

B, S, D, H, FF, L = 8, 1024, 512, 8, 2048, 4
DK = D // H          # 64
NKT = D // 128       # 4  feature tiles
NJT = S // 128       # 8  token tiles
NFT = FF // 128      # 16 ffn tiles
SCALE = 1.0 / math.sqrt(DK)
EPS = 1e-5
NCORES = 8

_PROG_CACHE = {}


def _strip_chunks(a):
    """Column chunks (absolute i ranges) for scores/PV strip of j-tile a:
    the 128-wide diagonal block first, then pieces that cross neither an
    absolute 512-boundary (PV psum banks) nor a strip-local one (scores
    psum banks, local = absolute - 128*a)."""
    chunks = [(128 * a, 128 * a + 128)]
    start = 128 * a + 128
    pts = sorted({512, 128 * a + 512, S})
    for p in pts:
        if start < p <= S:
            chunks.append((start, p))
            start = p
    return chunks


def _build(has_bv, bk_zero=True, ln1_triv=True, ln2_triv=True, b2_zero=True):
    nc = bacc.Bacc("TRN2", target_bir_lowering=False, debug=False,
                   num_devices=NCORES)

    xT_e = nc.declare_dram_parameter("xT", [D, S], BF16, isOutput=False)
    yh_e = nc.declare_dram_parameter("yTh", [D, S], F8, isOutput=False)
    yl_e = nc.declare_dram_parameter("yTl", [D, S], F8, isOutput=False)
    wk_e = nc.declare_dram_parameter("wkT", [L, D, D], BF16, isOutput=False)
    wvh_e = nc.declare_dram_parameter("wvTh", [L, D, D], F8, isOutput=False)
    wvl_e = nc.declare_dram_parameter("wvTl", [L, D, D], F8, isOutput=False)
    wo_e = nc.declare_dram_parameter("woT", [L, D, D], BF16, isOutput=False)
    w1_e = nc.declare_dram_parameter("w1T", [L, D, FF], BF16, isOutput=False)
    w2h_e = nc.declare_dram_parameter("w2Th", [L, FF, D], F8, isOutput=False)
    w2l_e = nc.declare_dram_parameter("w2Tl", [L, FF, D], F8, isOutput=False)
    # per-feature params packed [128, L, ntiles]
    bk_e = nc.declare_dram_parameter("bkp", [128, L, NKT], F32, isOutput=False)
    bo_e = nc.declare_dram_parameter("bop", [128, L, NKT], F32, isOutput=False)
    b1_e = nc.declare_dram_parameter("b1p", [128, L, NFT], F32, isOutput=False)
    b2_e = nc.declare_dram_parameter("b2p", [128, L, NKT], F32, isOutput=False)
    l1s_e = nc.declare_dram_parameter("l1s", [128, L, NKT], F32, isOutput=False)
    l1b_e = nc.declare_dram_parameter("l1b", [128, L, NKT], F32, isOutput=False)
    l2s_e = nc.declare_dram_parameter("l2s", [128, L, NKT], F32, isOutput=False)
    l2b_e = nc.declare_dram_parameter("l2b", [128, L, NKT], F32, isOutput=False)
    bv_e = nc.declare_dram_parameter("bvp", [1, L, D], BF16, isOutput=False) if has_bv else None
    mask_e = nc.declare_dram_parameter("mask01", [128, 128], BF16, isOutput=False)
    out_e = nc.declare_dram_parameter("outT", [D, S], F32, isOutput=True)

    with tile.TileContext(nc) as tc:
        with (
            tc.tile_pool(name="res", bufs=1) as res,         # resident activations
            tc.tile_pool(name="wqkv", bufs=2) as wqkv,       # per-layer D x D weights
            tc.tile_pool(name="wff", bufs=2) as wff,         # per-layer ffn weights
            tc.tile_pool(name="pt", bufs=5) as ptp,          # exp'd prob strips
            tc.tile_pool(name="vp", bufs=2) as vp,           # v_ext double buffer
            tc.tile_pool(name="rc", bufs=3) as rcp,          # per-head recip rows
            tc.tile_pool(name="bc", bufs=1) as bc,           # LN broadcast tiles
            tc.tile_pool(name="tb", bufs=2) as tbp,
            tc.tile_pool(name="h1p", bufs=1) as h1p,
            tc.tile_pool(name="wff2", bufs=1) as wff2,          # LN apply temps
            tc.tile_pool(name="ps", bufs=2, space="PSUM") as ps,
            tc.tile_pool(name="psh", bufs=4, space="PSUM") as psh,
            tc.tile_pool(name="rb", bufs=3) as rbp,          # recip bcast rows
            tc.tile_pool(name="dr", bufs=4, space="DRAM") as dr,
        ):
            # ---- residents (bf16 residual stream)
            x_bf = res.tile([128, NKT, S], BF16, tag="x_bf")     # post-LN x
            x2_bf = res.tile([128, NKT, S], BF16, tag="x2_bf")   # pre-LN resid
            y_hi = res.tile([128, NKT, S], F8, tag="y_hi")
            y_lo = res.tile([128, NKT, S], F8, tag="y_lo")
            kq_bf = res.tile([128, NKT, S], BF16, tag="kq_bf")
            outcat = res.tile([128, NKT, S], BF16, tag="outcat")
            # h1 fp8 hi/lo, one tile per 512-token chunk: keeps the Act-relu
            # (hi) and DVE-stt (lo) evictions of different chunks free of
            # false subtile WARs (tracking is first-free-dim granular)
            h1_hi = [h1p.tile([128, NFT, 512], F8, tag=f"h1h{c}", name=f"h1h{c}")
                     for c in range(2)]
            h1_lo = [h1p.tile([128, NFT, 512], F8, tag=f"h1l{c}", name=f"h1l{c}")
                     for c in range(2)]
            xsq_bf = res.tile([128, NKT, S], BF16, tag="xsq")

            # ---- initial loads: x = q+pe, y = qa+pe precomputed on host.
            # sync = HWDGE (fast, low latency): carries the startup-critical
            # loads in need order (y+wv feed vproj first), then only the
            # latency-sensitive softmax bounces + output stores.
            # gpsimd = SWDGE (Q7 descgen ~1us, latency-tolerant): params,
            # mask, and all next-layer weight prefetches.
            yh4 = yh_e.rearrange("(k p) s -> p k s", p=128)
            yl4 = yl_e.rearrange("(k p) s -> p k s", p=128)

            def ln_stats_mm(lp, kt, ch):
                """One accumulation step of the per-chunk stats sums; emit
                kt lagged behind the producer's evict of feature tile kt so
                it never head-of-line-blocks the producer's own matmuls."""
                lp0, lp1 = lp
                cs = slice(ch * 512, ch * 512 + 512)
                nc.tensor.matmul(lp0, lhsT=ones128, rhs=x2_bf[:, kt, cs],
                                 start=(kt == 0), stop=(kt == NKT - 1))
                nc.tensor.matmul(lp1, lhsT=ones128, rhs=xsq_bf[:, kt, cs],
                                 start=(kt == 0), stop=(kt == NKT - 1))

            def new_lp():
                return (psh.tile([128, 512], F32, tag="mmh", name="lp0"),
                        psh.tile([128, 512], F32, tag="mmh", name="lp1"))

            def ln_stats_fin(lp, ch, meanb, sbv):
                """Finish one chunk's stats: mean/E[x2] eviction split across
                Act (mean, mean^2) and DVE (E[x2], var) so the serial chain is
                ~1us shorter than an all-DVE version; rstd via Ln/Exp."""
                lp0, lp1 = lp
                cs = slice(ch * 512, ch * 512 + 512)
                nc.scalar.mul(meanb[:, cs], lp0, 1.0 / D)
                nc.vector.tensor_scalar_mul(sbv[:, cs], lp1, 1.0 / D)
                m2 = tbp.tile([128, 512], BF16, tag="lntmp")
                nc.scalar.square(m2, meanb[:, cs])
                nc.vector.scalar_tensor_tensor(out=sbv[:, cs], in0=sbv[:, cs],
                                               scalar=float(EPS), in1=m2,
                                               op0=ALU.add, op1=ALU.subtract)
                nc.scalar.activation(out=sbv[:, cs], in_=sbv[:, cs], func=AF.Ln)
                nc.scalar.activation(out=sbv[:, cs], in_=sbv[:, cs],
                                     func=AF.Exp, scale=-0.5)

            def ln_stats_ch(ch, meanb, sbv):
                lp = new_lp()
                for kt in range(NKT):
                    ln_stats_mm(lp, kt, ch)
                ln_stats_fin(lp, ch, meanb, sbv)

            def new_stats():
                meanb = bc.tile([128, S], BF16, tag="meanb")
                sbv = bc.tile([128, S], BF16, tag="statb")  # ex2->var->rstd
                return meanb, sbv

            oute4 = out_e.rearrange("(k p) s -> p k s", p=128)

            def ln_apply_ch(ch, meanb, sbv, lname_s, lname_b, li, triv,
                            final=False, xout_ch=None):
                """Apply one chunk's LN: x = (x2 - mean) * rstd (+affine).
                Non-final+trivial uses 2 coarse DVE ops; final writes the
                fp32 staging tile and DMAs per kt."""
                cs = slice(ch * 512, ch * 512 + 512)
                if not final and triv:
                    # pairs of kt per op: x(kt0,kt1) lands early enough for
                    # the consumer matmuls' first accumulation steps while
                    # keeping the DVE queue short
                    for kt in range(0, NKT, 2):
                        mb = meanb[:, None, cs].to_broadcast([128, 2, 512])
                        sb = sbv[:, None, cs].to_broadcast([128, 2, 512])
                        nc.vector.tensor_sub(xsq_bf[:, kt:kt + 2, cs],
                                             x2_bf[:, kt:kt + 2, cs], mb)
                        nc.vector.tensor_mul(x_bf[:, kt:kt + 2, cs],
                                             xsq_bf[:, kt:kt + 2, cs], sb)
                    return
                for kt in range(NKT):
                    nc.vector.tensor_sub(xsq_bf[:, kt, cs],
                                         x2_bf[:, kt, cs], meanb[:, cs])
                    if final:
                        po = xout_ch[:, kt, :]
                        nc.vector.tensor_mul(po, xsq_bf[:, kt, cs], sbv[:, cs])
                        if not triv:
                            nc.vector.tensor_scalar(
                                out=po, in0=po,
                                scalar1=params[lname_s][:, li, kt:kt + 1],
                                scalar2=params[lname_b][:, li, kt:kt + 1],
                                op0=ALU.mult, op1=ALU.add)
                        nc.sync.dma_start(out=oute4[:, kt, cs], in_=po)
                    else:
                        nc.vector.tensor_mul(x_bf[:, kt, cs],
                                             xsq_bf[:, kt, cs], sbv[:, cs])
                        if not triv:
                            nc.vector.tensor_scalar(
                                out=x_bf[:, kt, cs], in0=x_bf[:, kt, cs],
                                scalar1=params[lname_s][:, li, kt:kt + 1],
                                scalar2=params[lname_b][:, li, kt:kt + 1],
                                op0=ALU.mult, op1=ALU.add)

            def layernorm(lname_s, lname_b, li, triv, filler=None, stats=None):
                """LN over features of x2_bf -> x_bf. Stats from PE
                ones-matmul sums of the same bf16 values the apply uses; pass
                `stats`=(meanb, sbv) when the producing phase already emitted
                per-ch stats."""
                if stats is None:
                    meanb, sbv = new_stats()
                    for ch in range(2):
                        ln_stats_ch(ch, meanb, sbv)
                else:
                    meanb, sbv = stats
                if filler is not None:
                    filler()
                for ch in range(2):
                    ln_apply_ch(ch, meanb, sbv, lname_s, lname_b, li, triv)


            def vproj(li, wv, jts):
                """v = y @ WvT (token-major) -- depends only on y hi/lo + wv,
                so it can fill PE bubbles in other phases. fp8 DoubleRow,
                3-term split: psum holds 32*v, descaled at eviction."""
                wvh, wvl = wv
                vx4 = _vx4_of[li]
                nc.gpsimd.memset(vx4[:, jts.start:jts.stop, :, 64:65], 1.0)
                for jt in jts:
                    pp = ps.tile([128, S], F32, tag="mm")
                    for half in range(2):
                        c0 = half * 256
                        sl = pp[:, c0:c0 + 256]
                        seq = [(y_hi, wvh, 0), (y_hi, wvh, 2),
                               (y_hi, wvl, 0), (y_hi, wvl, 2),
                               (y_lo, wvh, 0), (y_lo, wvh, 2)]
                        for i, (ya, wa, ktp) in enumerate(seq):
                            nc.tensor.matmul(
                                sl, lhsT=ya[:, ktp:ktp + 2, jt * 128:jt * 128 + 128],
                                rhs=wa[:, ktp:ktp + 2, c0:c0 + 256],
                                start=(i == 0),
                                stop=(i == len(seq) - 1) and not has_bv,
                                perf_mode=DR)
                        if has_bv:
                            nc.tensor.matmul(sl, lhsT=ones_row,
                                             rhs=bv_sb[0:1, li, c0:c0 + 256],
                                             start=False, stop=True,
                                             skip_group_check=True)
                    nc.scalar.mul(
                        vx4[:, jt, :, 0:64],
                        pp[:, 0:512].rearrange("p (h c) -> p h c", c=64),
                        1.0 / WV_SCL)
                return vx4

            _vx4_of = {}

            def new_vext(li):
                vext = vp.tile([128, NJT, H * 72], BF16, tag="vext")
                _vx4_of[li] = vext.rearrange("p j (h c) -> p j h c", c=72)
                return _vx4_of[li]

            wv_tiles = {}
            wk_tiles = {}
            wo_tiles = {}

            def load_wv(li, eng):
                th = wqkv.tile([128, NKT, D], F8, tag="wvh")
                eng.dma_start(out=th, in_=wvh_e[li].rearrange("(k p) m -> p k m", p=128))
                tl = wqkv.tile([128, NKT, D], F8, tag="wvl")
                eng.dma_start(out=tl, in_=wvl_e[li].rearrange("(k p) m -> p k m", p=128))
                return th, tl

            def load_wk_wo(li, eng):
                wk = wqkv.tile([128, NKT, D], BF16, tag="wk")
                eng.dma_start(out=wk, in_=wk_e[li].rearrange("(k p) m -> p k m", p=128))
                wo = wqkv.tile([128, NKT, D], BF16, tag="wo")
                eng.dma_start(out=wo, in_=wo_e[li].rearrange("(k p) m -> p k m", p=128))
                return wk, wo

            wff_tiles = {}

            def load_wff(li, eng):
                w1 = wff.tile([128, NKT, FF], BF16, tag="w1")
                eng.dma_start(out=w1, in_=w1_e[li].rearrange("(k p) m -> p k m", p=128))
                w2h = wff2.tile([128, NFT, D], F8, tag="w2h")
                eng.dma_start(out=w2h, in_=w2h_e[li].rearrange("(k p) m -> p k m", p=128))
                w2l = wff2.tile([128, NFT, D], F8, tag="w2l")
                eng.dma_start(out=w2l, in_=w2l_e[li].rearrange("(k p) m -> p k m", p=128))
                return w1, (w2h, w2l)

            nc.sync.dma_start(out=y_hi[:, :, 0:512], in_=yh4[:, :, 0:512])
            nc.sync.dma_start(out=y_lo[:, :, 0:512], in_=yl4[:, :, 0:512])
            wv_tiles[0] = load_wv(0, nc.sync)
            nc.sync.dma_start(out=y_hi[:, :, 512:1024], in_=yh4[:, :, 512:1024])
            nc.sync.dma_start(out=y_lo[:, :, 512:1024], in_=yl4[:, :, 512:1024])
            xT4 = xT_e.rearrange("(k p) s -> p k s", p=128)
            nc.sync.dma_start(out=x_bf[:, :, 0:512], in_=xT4[:, :, 0:512])
            nc.sync.dma_start(out=x_bf[:, :, 512:1024], in_=xT4[:, :, 512:1024])
            wk_tiles[0], wo_tiles[0] = load_wk_wo(0, nc.sync)
            wff_tiles[0] = load_wff(0, nc.sync)

            # ---- constants & params (issued after the critical-path loads)
            params = {}
            mask01 = res.tile([128, 128], BF16, tag="mask")
            nc.gpsimd.dma_start(out=mask01, in_=mask_e[:])
            ones128 = res.tile([128, 128], BF16, tag="ones")
            nc.vector.memset(ones128, 1.0)
            ones_1x64 = res.tile([1, 64], BF16, tag="ones64")
            nc.vector.memset(ones_1x64, 1.0)
            if has_bv:
                ones_row = res.tile([1, 128], BF16, tag="onesr")
                nc.vector.memset(ones_row, 1.0)
                bv_sb = res.tile([1, L, D], BF16, tag="bv")
                nc.gpsimd.dma_start(out=bv_sb, in_=bv_e[:])
            for name, ext, nt in (("bk", bk_e, NKT), ("bo", bo_e, NKT),
                                  ("b1", b1_e, NFT), ("b2", b2_e, NKT),
                                  ("l1s", l1s_e, NKT), ("l1b", l1b_e, NKT),
                                  ("l2s", l2s_e, NKT), ("l2b", l2b_e, NKT)):
                t = res.tile([128, L, nt], F32, tag="prm_" + name)
                nc.gpsimd.dma_start(out=t, in_=ext[:])
                params[name] = t

            new_vext(0)
            vx4 = vproj(0, wv_tiles[0], range(0, NJT))

            for li in range(L):
                wk, wo = wk_tiles[li], wo_tiles[li]
                w1, (w2h, w2l) = wff_tiles[li]
                if li + 1 < L:
                    new_vext(li + 1)
                    # wv early on the same HWDGE queue: its transfer lands
                    # during kq-proj, before the first softmax bounce
                    wv_tiles[li + 1] = load_wv(li + 1, nc.sync)

                # ---- kq projection tile mt feeds heads 2mt, 2mt+1:
                # interleave so exps start after one kq tile and later kq
                # tiles keep the PE busy under the early heads' exps
                # kq proj in ch-outer per-(ch, mt) psh pieces: the ch0 pieces
                # run on the PE while the LN2 ch1 apply still drains on DVE.
                # DVE eviction keeps the Act queue clear for head-0 exps.
                for ch in range(2):
                    c0 = ch * 512
                    for mt in range(NKT):
                        pf = psh.tile([128, 512], F32, tag="mmh", name="pk")
                        for kt in range(NKT):
                            nc.tensor.matmul(pf,
                                             lhsT=wk[:, kt, mt * 128:mt * 128 + 128],
                                             rhs=x_bf[:, kt, c0:c0 + 512],
                                             start=(kt == 0), stop=(kt == NKT - 1))
                        if bk_zero:
                            nc.vector.tensor_copy(kq_bf[:, mt, c0:c0 + 512], pf)
                        else:
                            nc.vector.tensor_scalar_add(
                                kq_bf[:, mt, c0:c0 + 512], pf,
                                params["bk"][:, li, mt:mt + 1])

                # ---- attention, head by head
                for h in range(H):
                    po = (h % 2) * 64
                    kqh = kq_bf[po:po + 64, h // 2, :]
                    acc0 = psh.tile([128, 512], F32, tag="mmh")
                    acc1 = psh.tile([128, 512], F32, tag="mmh")
                    # strip groups share one psum tile + one exp: strips
                    # 4..7 are narrow enough to pack pairwise with no padding
                    # (local offsets chosen so no matmul crosses a psum bank)
                    def emit_group(group, locs):
                        sc = ps.tile([128, S], F32, tag="mm")
                        for a in group:
                            for (s0, e0) in _strip_chunks(a):
                                lo = locs[a] + s0 - 128 * a
                                nc.tensor.matmul(sc[:, lo:lo + (e0 - s0)],
                                                 lhsT=kqh[:, 128 * a:128 * a + 128],
                                                 rhs=kqh[:, s0:s0 + (e0 - s0)],
                                                 start=True, stop=True)
                        wtot = max(locs[a] + S - 128 * a for a in group)
                        pt = ptp.tile([128, S], BF16, tag="pt")
                        nc.scalar.activation(out=pt[:, 0:wtot], in_=sc[:, 0:wtot],
                                             func=AF.Exp, scale=float(SCALE))
                        if (len(group) == 2
                                and locs[group[1]] - locs[group[0]] <= 512):
                            stride = locs[group[1]] - locs[group[0]]
                            view = pt[:, 0:stride * len(group)].rearrange(
                                "p (g c) -> p g c", c=stride)
                            nc.vector.tensor_mul(view[:, :, 0:128], view[:, :, 0:128],
                                                 mask01[:, None, :].to_broadcast(
                                                     [128, len(group), 128]))
                        else:
                            for a in group:
                                lo = locs[a]
                                nc.vector.tensor_mul(pt[:, lo:lo + 128],
                                                     pt[:, lo:lo + 128], mask01)
                        return pt

                    def emit_pv(group, locs, pt):
                        # PV needs only the diagonal + absolute-512 splits
                        # (strip-local splits are a scores-psum constraint)
                        for a in group:
                            chunks = [(128 * a, 128 * a + 128)]
                            st = 128 * a + 128
                            for p in (512, S):
                                if st < p <= S:
                                    chunks.append((st, p))
                                    st = p
                            for (s0, e0) in chunks:
                                acc = acc0 if s0 < 512 else acc1
                                o0 = s0 - (0 if s0 < 512 else 512)
                                lo = locs[a] + s0 - 128 * a
                                nc.tensor.matmul(acc[0:65, o0:o0 + (e0 - s0)],
                                                 lhsT=vx4[:, a, h, 0:65],
                                                 rhs=pt[:, lo:lo + (e0 - s0)],
                                                 start=(a == 0),
                                                 stop=(e0 == 128 * a + 128),
                                                 skip_group_check=True)

                    # softmax denominators: bf16 reciprocal straight from the
                    # psum ones-row, col 0 zeroed (empty first query row), then
                    # a DRAM-bounce DMA (HWDGE, low latency) broadcasts it
                    # across 64 partitions and the eviction multiply
                    # normalizes straight out of psum (TT ops may read at most
                    # ONE operand from PSUM, so the broadcast side must be
                    # SBUF). acc0 (cols 0..512) finishes one PV group before
                    # acc1, so its bounce overlaps the last PV group.
                    recip = rcp.tile([1, S], BF16, tag="recip")
                    bnc = dr.tile([1, S], BF16, tag="bounce")
                    rbh = rbp.tile([64, S], BF16, tag="rbh")

                    def norm_half(c0):
                        acc = acc0 if c0 == 0 else acc1
                        cs = slice(c0, c0 + 512)
                        with nc.allow_low_precision(reason="softmax 1/l bf16"):
                            nc.vector.reciprocal(recip[:, cs], acc[64:65, :])
                        if c0 == 0:
                            nc.gpsimd.memset(recip[:, 0:1], 0.0)
                        if h >= H - 2:
                            # last head: local broadcast (PE matmul -> psum
                            # -> SBUF copy) instead of the DRAM bounce; the
                            # ~5us DMA round-trip would be fully exposed at
                            # the phase boundary
                            bps = psh.tile([128, 512], F32, tag="mmh")
                            nc.tensor.matmul(bps[0:64, :], lhsT=ones_1x64,
                                             rhs=recip[0:1, cs],
                                             start=True, stop=True)
                            nc.vector.tensor_copy(rbh[:, cs], bps[0:64, :])
                        else:
                            nc.sync.dma_start(out=bnc[:, cs], in_=recip[:, cs])
                            bap = bnc[:, cs]
                            nc.sync.dma_start(out=rbh[:, cs], in_=bass.AP(
                                tensor=bap.tensor, offset=bap.offset,
                                ap=[[0, 64]] + bap.ap[1:]))

                    def evict_half(c0):
                        acc = acc0 if c0 == 0 else acc1
                        cs = slice(c0, c0 + 512)
                        nc.vector.tensor_mul(outcat[po:po + 64, h // 2, cs],
                                             acc[0:64, :], rbh[:, cs])

                    GROUPS = [([0], {0: 0}), ([1], {1: 0}),
                              ([2, 6], {2: 0, 6: 768}),
                              ([3, 7], {3: 0, 7: 640}),
                              ([4, 5], {4: 0, 5: 512})]
                    pts = [emit_group(*GROUPS[0]), emit_group(*GROUPS[1])]
                    emit_pv(*GROUPS[0], pts[0])
                    pts.append(emit_group(*GROUPS[2]))
                    emit_pv(*GROUPS[1], pts[1])
                    pts.append(emit_group(*GROUPS[3]))
                    emit_pv(*GROUPS[2], pts[2])
                    pts.append(emit_group(*GROUPS[4]))
                    emit_pv(*GROUPS[3], pts[3])
                    norm_half(0)       # acc0 complete: strips 4..7 are acc1-only
                    emit_pv(*GROUPS[4], pts[4])
                    evict_half(0)
                    norm_half(512)
                    evict_half(512)

                # prefetch next layer's weights HERE, on the same in-order
                # HWDGE queue as the softmax bounces: emitted after the head
                # loop, the transfers are forced to queue behind every
                # bounce (cross-queue issue order is NOT program order - the
                # idle Pool sequencer races ahead and its transfers would
                # otherwise block head 0's bounce on the serial DMA engine)
                if li + 1 < L:
                    wk_tiles[li + 1], wo_tiles[li + 1] = load_wk_wo(li + 1, nc.sync)
                    wff_tiles[li + 1] = load_wff(li + 1, nc.sync)
                # last-head bounce filler: two next-layer v-proj token tiles
                if li + 1 < L:
                    vproj(li + 1, wv_tiles[li + 1], range(0, 2))

                # ---- out projection + residual into x2_bf, ch-outer with
                # per-(ch, mt) psum pieces: chunk-0 LN1 stats overlap chunk-1
                # matmuls, hiding most of the LN chain
                ln1_st = new_stats()
                for ch in range(2):
                    c0 = ch * 512
                    cs = slice(c0, c0 + 512)
                    lp = new_lp()
                    for mt in range(NKT):
                        pf = psh.tile([128, 512], F32, tag="mmh", name="po")
                        for kt in range(NKT):
                            nc.tensor.matmul(pf,
                                             lhsT=wo[:, kt, mt * 128:mt * 128 + 128],
                                             rhs=outcat[:, kt, cs],
                                             start=(kt == 0), stop=(kt == NKT - 1))
                        nc.vector.scalar_tensor_tensor(
                            out=x2_bf[:, mt, cs], in0=pf,
                            scalar=params["bo"][:, li, mt:mt + 1],
                            in1=x_bf[:, mt, cs], op0=ALU.add, op1=ALU.add)
                        # square immediately: keeps the LN1 stats chain short
                        nc.scalar.activation(out=xsq_bf[:, mt, cs],
                                             in_=x2_bf[:, mt, cs], func=AF.Square)
                    for kt in range(NKT):
                        ln_stats_mm(lp, kt, ch)
                    ln_stats_fin(lp, ch, *ln1_st)

                fil1 = ((lambda: vproj(li + 1, wv_tiles[li + 1], range(2, 5)))
                        if li + 1 < L else None)
                layernorm("l1s", "l1b", li, ln1_triv, filler=fil1, stats=ln1_st)
                # ffn1-head filler: vproj uses ps-pool psums (disjoint from
                # ffn1's psh pieces) and keeps the PE warm while the LN apply
                # + first h1 evictions drain on DVE/Act
                if li + 1 < L:
                    vproj(li + 1, wv_tiles[li + 1], range(5, 7))

                # ---- ffn1: h1 = relu(W1 @ x + b1), feature-major (bf16
                # matmuls). Evicted directly as an fp8 hi/lo pair for ffn2's
                # DoubleRow matmuls: hi = Act relu (fp8), lo = relu(psum) - hi
                # on DVE. Each (mt, ch) gets its own 1-bank psum piece from
                # psh: psum readers are chained in emission order, so small
                # pieces keep the relu->stt chain at 2 ops and release banks
                # early (the ps-pool [128,S] tiles would serialize 4 readers).
                for ch in range(2):
                    for mt in range(NFT):
                        c0 = ch * 512
                        pf = psh.tile([128, 512], F32, tag="mmh", name="pf")
                        for kt in range(NKT):
                            nc.tensor.matmul(pf,
                                             lhsT=w1[:, kt, mt * 128:mt * 128 + 128],
                                             rhs=x_bf[:, kt, c0:c0 + 512],
                                             start=(kt == 0), stop=(kt == NKT - 1))
                        nc.scalar.activation(out=h1_hi[ch][:, mt, :],
                                             in_=pf, func=AF.Relu,
                                             bias=params["b1"][:, li, mt:mt + 1])
                        nc.vector.scalar_tensor_tensor(
                            out=h1_lo[ch][:, mt, :], in0=pf, scalar=0.0,
                            in1=h1_hi[ch][:, mt, :], op0=ALU.max,
                            op1=ALU.subtract)

                # ---- ffn2 + residual into x2_bf, ch-outer with per-(ch, mt)
                # psum pieces. fp8 DoubleRow 3-term; psum = 2048*ff, descaled
                # in the eviction stt. Chunk-0 LN2 stats (and, on the final
                # layer, chunk-0 apply + output stores) overlap chunk-1.
                final = (li == L - 1)
                ln2_st = new_stats()
                for ch in range(2):
                    cs = slice(ch * 512, ch * 512 + 512)
                    lp = new_lp()
                    for mt in range(NKT):
                        pf = psh.tile([128, 512], F32, tag="mmh", name="pf2")
                        for half in range(2):
                            ci = half * 256
                            sl = pf[:, ci:ci + 256]
                            first = True
                            for ha, wa in ((h1_hi[ch], w2h), (h1_hi[ch], w2l),
                                           (h1_lo[ch], w2h)):
                                for ktp in range(0, NFT, 2):
                                    nc.tensor.matmul(
                                        sl,
                                        lhsT=wa[:, ktp:ktp + 2,
                                                mt * 128:mt * 128 + 128],
                                        rhs=ha[:, ktp:ktp + 2, ci:ci + 256],
                                        start=first,
                                        stop=(ha is h1_lo[ch] and ktp == NFT - 2),
                                        perf_mode=DR)
                                    first = False
                        nc.vector.scalar_tensor_tensor(
                            out=x2_bf[:, mt, cs], in0=pf,
                            scalar=1.0 / W2_SCL,
                            in1=x_bf[:, mt, cs], op0=ALU.mult, op1=ALU.add)
                        if not b2_zero:
                            nc.vector.tensor_scalar_add(
                                x2_bf[:, mt, cs], x2_bf[:, mt, cs],
                                params["b2"][:, li, mt:mt + 1])
                        # square immediately: keeps the LN2 stats chain short
                        nc.scalar.activation(out=xsq_bf[:, mt, cs],
                                             in_=x2_bf[:, mt, cs], func=AF.Square)
                    for kt in range(NKT):
                        ln_stats_mm(lp, kt, ch)
                    ln_stats_fin(lp, ch, *ln2_st)
                    if final:
                        # stream this chunk straight out while the other
                        # chunk's ffn2 matmuls keep the PE busy
                        xout_ch = h1p.tile([128, NKT, 512], F32,
                                           tag=f"h1l{ch}", name=f"xout{ch}")
                        ln_apply_ch(ch, *ln2_st, "l2s", "l2b", li, ln2_triv,
                                    final=True, xout_ch=xout_ch)

                if not final:
                    fil2 = ((lambda: vproj(li + 1, wv_tiles[li + 1],
                                           range(7, NJT)))
                            if li + 1 < L else None)
                    layernorm("l2s", "l2b", li, ln2_triv, filler=fil2,
                              stats=ln2_st)
                    vx4 = _vx4_of[li + 1]

    # Pin every ACT instruction to the one table set that contains all the
    # functions this kernel uses (Exp/Ln/Identity/Relu/Square/Copy), so the
    # whole kernel needs a single ACT_TABLE_LOAD instead of thrashing between
    # the exp- and ln-anchored sets on every layernorm. Indices are preserved
    # (the pass emits act_func_set_id by list position).
    import concourse.bacc as _bacc_mod
    _orig_gat = _bacc_mod.get_activation_tables
    def _pinned_tables(arch):
        tabs = _orig_gat(arch)
        return {name: (funcs if name == "natural_log_exp_and_others" else set())
                for name, funcs in tabs.items()}
    _bacc_mod.get_activation_tables = _pinned_tables
    try:
        nc.compile()
    finally:
        _bacc_mod.get_activation_tables = _orig_gat
    return nc


def _pack_feat(arr, nt):
    """(L, nt*128) fp32 -> [128, L, nt]"""
    Ld = arr.shape[0]
    return np.ascontiguousarray(arr.reshape(Ld, nt, 128).transpose(2, 0, 1)).astype(np.float32)


def _split_f8(a, scale):
    """Split scale*a into fp8e4 hi + lo (hi = rne(x), lo = rne(x - hi))."""
    f8 = ml_dtypes.float8_e4m3
    x = np.clip(np.asarray(a, np.float32) * scale, -240.0, 240.0)
    hi = x.astype(f8)
    lo = np.clip(x - hi.astype(np.float32), -240.0, 240.0).astype(f8)
    return np.ascontiguousarray(hi), np.ascontiguousarray(lo)


def kernel(q_embed_data, qa_embed_data, pe, Wk, bk, Wv, bv, Wo, bo,
           ln1_s, ln1_b, W1, b1, W2, b2, ln2_s, ln2_b, **_unused):
    q = np.asarray(q_embed_data, np.float32)
    qa = np.asarray(qa_embed_data, np.float32)
    pe = np.asarray(pe, np.float32)
    bf = ml_dtypes.bfloat16

    has_bv = bool(np.any(np.asarray(bv)))
    bk_zero = not bool(np.any(np.asarray(bk)))
    b2_zero = not bool(np.any(np.asarray(b2)))
    ln1_triv = bool(np.all(np.asarray(ln1_s) == 1.0) and not np.any(np.asarray(ln1_b)))
    ln2_triv = bool(np.all(np.asarray(ln2_s) == 1.0) and not np.any(np.asarray(ln2_b)))
    key = (has_bv, bk_zero, ln1_triv, ln2_triv, b2_zero)
    if key not in _PROG_CACHE:
        _PROG_CACHE[key] = _build(has_bv, bk_zero, ln1_triv, ln2_triv, b2_zero)
    nc = _PROG_CACHE[key]

    wvT = np.asarray(Wv, np.float32).transpose(0, 2, 1)
    w2T = np.asarray(W2, np.float32).transpose(0, 2, 1)
    wvh, wvl = _split_f8(wvT, WV_SCL)
    w2h, w2l = _split_f8(w2T, W2_SCL)
    shared = {
        "wkT": np.ascontiguousarray(np.asarray(Wk, np.float32).transpose(0, 2, 1)).astype(bf),
        "wvTh": wvh, "wvTl": wvl,
        "woT": np.ascontiguousarray(np.asarray(Wo, np.float32).transpose(0, 2, 1)).astype(bf),
        "w1T": np.ascontiguousarray(np.asarray(W1, np.float32).transpose(0, 2, 1)).astype(bf),
        "w2Th": w2h, "w2Tl": w2l,
        "bkp": _pack_feat(np.asarray(bk, np.float32), NKT),
        "bop": _pack_feat(np.asarray(bo, np.float32), NKT),
        "b1p": _pack_feat(np.asarray(b1, np.float32), NFT),
        "b2p": _pack_feat(np.asarray(b2, np.float32), NKT),
        "l1s": _pack_feat(np.asarray(ln1_s, np.float32), NKT),
        "l1b": _pack_feat(np.asarray(ln1_b, np.float32), NKT),
        "l2s": _pack_feat(np.asarray(ln2_s, np.float32), NKT),
        "l2b": _pack_feat(np.asarray(ln2_b, np.float32), NKT),
        "mask01": (np.arange(128)[:, None] < np.arange(128)[None, :]).astype(bf),
    }
    if has_bv:
        shared["bvp"] = (np.asarray(bv, np.float32) * WV_SCL).reshape(1, L, D).astype(bf)

    in_maps = []
    for c in range(NCORES):
        m = dict(shared)
        m["xT"] = np.ascontiguousarray((q[c] + pe).T).astype(bf)
        yh, yl = _split_f8((qa[c] + pe).T, 1.0)
        m["yTh"] = yh
        m["yTl"] = yl
        in_maps.append(m)

    res = run_bass_kernel_spmd(nc, in_maps, core_ids=list(range(NCORES)))
    out = np.stack([np.ascontiguousarray(res.results[c]["outT"].T)
                    for c in range(NCORES)])
    return out.astype(np.float32)

